# revision 1
# baseline (speedup 1.0000x reference)
"""CTGRU forward kernel for 8 trn2 NeuronCores (data-parallel over batch).

Layout on device (per core, local batch BL=512):
  - All per-step tensors live as [U_partitions, m*BL + b] ("layout C", m-major
    free dim), so the hidden state h comes out of the m-reduction already in
    the [U, B] orientation the next step's matmuls need as their moving
    operand -> zero transposes inside the recurrence.
  - softmax(-(z - LN_TAU)^2) is computed as Derivative_Erf(z + (b - LN_TAU))
    = (2/sqrt(pi)) * exp(-d^2); the constant cancels in the normalization.
  - DECAY[0] == 0 exactly, so h_hat[..., m=0] is identically zero: the state
    stores only m=1..7 (7 slices) and all elementwise work skips m=0.
"""

import contextlib
import ctypes
import sys
import types

import numpy as np

B, T, F, U, M = 4096, 16, 256, 512, 8
N_CORES = 8
BL = B // N_CORES  # 512
KT = (F + U) // 128  # 6 K-tiles over the fused dim
UT = U // 128  # 4 u-tiles

LN_TAU = (np.arange(M, dtype=np.float32) * (0.5 * np.log(10.0))).astype(np.float32)
DECAY = np.exp(-0.04 / (LN_TAU + 1e-7)).astype(np.float32)  # DECAY[0] == 0.0


def _install_axon_hooks_shim():
    """Make `antenv.axon_hooks` importable when the image lacks it, so
    BASS_TRACE-triggered profiling in run_bass_kernel_spmd can't crash us."""
    name = "antenv.axon_hooks"
    if name in sys.modules:
        return
    so_path = "/opt/axon/libaxon_pjrt.so"

    def _build_hook():
        try:
            lib = ctypes.CDLL(so_path)
        except OSError:
            return None
        if not hasattr(lib, "axon_start_nrt_profile"):
            return None
        lib.axon_start_nrt_profile.argtypes = [
            ctypes.POINTER(ctypes.c_int64),
            ctypes.c_size_t,
        ]
        lib.axon_start_nrt_profile.restype = ctypes.c_int64
        lib.axon_stop_nrt_profile.argtypes = [ctypes.c_char_p]
        lib.axon_stop_nrt_profile.restype = ctypes.c_int64

        @contextlib.contextmanager
        def _hook(output_dir, device_ids):
            import jax

            jax.devices()
            if device_ids:
                ids = (ctypes.c_int64 * len(device_ids))(*device_ids)
                rc = lib.axon_start_nrt_profile(ids, len(device_ids))
            else:
                rc = lib.axon_start_nrt_profile(None, 0)
            if rc != 0:
                raise RuntimeError(f"axon_start_nrt_profile rc={rc}")
            try:
                yield
            finally:
                n = lib.axon_stop_nrt_profile(str(output_dir).encode())
                print(f"profile: {n} file(s) written to {output_dir}", file=sys.stderr)

        return _hook

    mod = types.ModuleType(name)
    holder = [_build_hook()]
    mod.get_axon_ntff_profile_hook = lambda: holder[0]
    mod.set_axon_ntff_profile_hook = lambda h: holder.__setitem__(0, h)
    sys.modules[name] = mod
    try:
        import antenv

        antenv.axon_hooks = mod
    except ImportError:
        pass


_program_cache = {}


def _build_program():
    if "nc" in _program_cache:
        return _program_cache["nc"]

    import concourse.bass as bass
    import concourse.tile as tile
    from concourse import bacc, mybir

    f32 = mybir.dt.float32
    bf16 = mybir.dt.bfloat16
    AF = mybir.ActivationFunctionType
    ALU = mybir.AluOpType
    AX = mybir.AxisListType

    nc = bacc.Bacc("TRN2", target_bir_lowering=False, debug=False)

    xt_d = nc.dram_tensor("xt", [T, 128, 2 * BL], bf16, kind="ExternalInput").ap()
    wr_d = nc.dram_tensor("wr", [128, M * UT * KT * 128], bf16, kind="ExternalInput").ap()
    ws_d = nc.dram_tensor("ws", [128, M * UT * KT * 128], bf16, kind="ExternalInput").ap()
    wq_d = nc.dram_tensor("wq", [128, UT * KT * 128], bf16, kind="ExternalInput").ap()
    wo_d = nc.dram_tensor("wo", [128, UT * 3], f32, kind="ExternalInput").ap()
    br_d = nc.dram_tensor("biasr", [128, UT * M], f32, kind="ExternalInput").ap()
    bs_d = nc.dram_tensor("biass", [128, UT * M], f32, kind="ExternalInput").ap()
    bq_d = nc.dram_tensor("biasq", [128, UT], f32, kind="ExternalInput").ap()
    y_d = nc.dram_tensor("y", [T, 3, BL], f32, kind="ExternalOutput").ap()

    with tile.TileContext(nc) as tc, contextlib.ExitStack() as ctx:
        const = ctx.enter_context(tc.tile_pool(name="const", bufs=1))
        state = ctx.enter_context(tc.tile_pool(name="state", bufs=1))
        wsp = ctx.enter_context(tc.tile_pool(name="wsp", bufs=2))
        rtp = ctx.enter_context(tc.tile_pool(name="rtp", bufs=2))
        xp = ctx.enter_context(tc.tile_pool(name="xp", bufs=2))
        ep = ctx.enter_context(tc.tile_pool(name="ep", bufs=2))
        tp = ctx.enter_context(tc.tile_pool(name="tp", bufs=2))
        wpp = ctx.enter_context(tc.tile_pool(name="wpp", bufs=2))
        sp = ctx.enter_context(tc.tile_pool(name="sp", bufs=2))
        stp = ctx.enter_context(tc.tile_pool(name="stp", bufs=2))
        pmm = ctx.enter_context(tc.tile_pool(name="pmm", bufs=4, space="PSUM"))
        pq = ctx.enter_context(tc.tile_pool(name="pq", bufs=2, space="PSUM"))
        py = ctx.enter_context(tc.tile_pool(name="py", bufs=2, space="PSUM"))

        # ---- weight / bias preload ----
        wr_sb = const.tile([128, M * UT * KT * 128], bf16, name="wr_sb")
        for m in range(M):
            sl = slice(m * UT * KT * 128, (m + 1) * UT * KT * 128)
            nc.sync.dma_start(wr_sb[:, sl], wr_d[:, sl])
        wq_sb = const.tile([128, UT * KT * 128], bf16, name="wq_sb")
        nc.sync.dma_start(wq_sb[:], wq_d[:])
        wo_sb = const.tile([128, UT * 3], f32, name="wo_sb")
        nc.sync.dma_start(wo_sb[:], wo_d[:])
        br_sb = const.tile([128, UT * M], f32, name="br_sb")
        nc.sync.dma_start(br_sb[:], br_d[:])
        bs_sb = const.tile([128, UT * M], f32, name="bs_sb")
        nc.sync.dma_start(bs_sb[:], bs_d[:])
        bq_sb = const.tile([128, UT], f32, name="bq_sb")
        nc.sync.dma_start(bq_sb[:], bq_d[:])

        # h_hat state: m-slices 1..7 only (slice 0 is identically zero)
        hhat = [
            state.tile([128, 7 * BL], bf16, name=f"hhat{u}", tag=f"hhat{u}")
            for u in range(UT)
        ]

        hT = None  # [128, UT*BL] bf16, h(t) transposed — rhs k-tiles for h-part

        def tree_reduce(src, n_m, out, eng=None):
            """out[128,BL] f32 = sum over n_m contiguous BL-slices of src.

            Contiguous TT adds instead of a strided inner-m reduce (the
            strided form runs ~0.5 elem/cycle on DVE). fp32 partials.
            """
            t1 = rtp.tile([128, 4 * BL], f32, name="rt1", tag="rt1")
            t2 = rtp.tile([128, 2 * BL], f32, name="rt2", tag="rt2")
            if eng is None:
                eng = nc.vector
            if n_m == 8:
                eng.tensor_add(t1[:], src[:, : 4 * BL], src[:, 4 * BL :])
                eng.tensor_add(t2[:], t1[:, : 2 * BL], t1[:, 2 * BL :])
                eng.tensor_add(out[:], t2[:, :BL], t2[:, BL:])
            else:  # 7 slices: (0..2)+(4..6), then pairs, + slice 3
                eng.tensor_add(
                    t1[:, : 3 * BL], src[:, : 3 * BL], src[:, 4 * BL : 7 * BL]
                )
                eng.tensor_add(t2[:, :BL], t1[:, :BL], t1[:, BL : 2 * BL])
                eng.tensor_add(
                    t2[:, BL : 2 * BL], t1[:, 2 * BL : 3 * BL], src[:, 3 * BL : 4 * BL]
                )
                eng.tensor_add(out[:], t2[:, :BL], t2[:, BL : 2 * BL])

        def mm_group(ps, w_sb, base, rhs_x, rhs_h, with_h):
            """Accumulate the K=768 fused matmul into psum `ps`."""
            nc.tensor.matmul(
                ps[:], w_sb[:, base : base + 128], rhs_x[:, 0:BL],
                start=True, stop=False,
            )
            nc.tensor.matmul(
                ps[:], w_sb[:, base + 128 : base + 256], rhs_x[:, BL : 2 * BL],
                start=False, stop=not with_h,
            )
            if with_h:
                for k in range(2, KT):
                    nc.tensor.matmul(
                        ps[:],
                        w_sb[:, base + k * 128 : base + (k + 1) * 128],
                        rhs_h[:, (k - 2) * BL : (k - 1) * BL],
                        start=False, stop=(k == KT - 1),
                    )

        for t in range(T):
            xt_t = xp.tile([128, 2 * BL], bf16, name="xt_t")
            nc.sync.dma_start(xt_t[:], xt_d[t])

            # ---------------- r phase (t=0: h_hat==0 makes r irrelevant) ----
            if t > 0:
                rh_bf = stp.tile([128, UT * BL], bf16, name="rh_bf")
                for u in range(UT):
                    e_r = ep.tile([128, M * BL], bf16, name="e_t", tag="e_t")
                    for m in range(M):
                        ps = pmm.tile([128, BL], f32, name="ps_mm", tag="ps_mm")
                        base = (m * UT + u) * KT * 128
                        mm_group(ps, wr_sb, base, xt_t, hT, True)
                        nc.scalar.activation(
                            e_r[:, m * BL : (m + 1) * BL], ps[:],
                            AF.Derivative_Erf,
                            bias=br_sb[:, u * M + m : u * M + m + 1],
                        )
                    denr = sp.tile([128, BL], f32, name="den", tag="den")
                    tree_reduce(e_r, 8, denr)
                    cr = sp.tile([128, BL], f32, name="crec", tag="crec")
                    nc.vector.reciprocal_approx_fast(out=cr[:], in_=denr[:])
                    eh = tp.tile([128, 7 * BL], bf16, name="ehtq", tag="ehtq")
                    nc.gpsimd.tensor_mul(eh[:], e_r[:, BL:], hhat[u][:])
                    rhn = sp.tile([128, BL], f32, name="rhn", tag="rhn")
                    tree_reduce(eh, 7, rhn)
                    nc.vector.tensor_mul(
                        rh_bf[:, u * BL : (u + 1) * BL], rhn[:], cr[:]
                    )

            # ------- s phase matmuls / elementwise, interleaved with q -----
            # PE order: [r], s_mm(0), q, s_mm(1), s_el(0), s_mm(2), s_el(1),
            # ... so PE never stalls waiting for the r-phase elementwise tail.
            h_f32 = stp.tile([128, UT * BL], f32, name="h_f32", bufs=1)

            def s_mm(u):
                e_s = ep.tile([128, M * BL], bf16, name="e_t", tag="e_t")
                for m in range(M):
                    wsch = wsp.tile([128, KT * 128], bf16, name="wsch")
                    base = (m * UT + u) * KT * 128
                    nc.sync.dma_start(wsch[:], ws_d[:, base : base + KT * 128])
                    ps = pmm.tile([128, BL], f32, name="ps_mm", tag="ps_mm")
                    mm_group(ps, wsch, 0, xt_t, hT, t > 0)
                    nc.scalar.activation(
                        e_s[:, m * BL : (m + 1) * BL], ps[:],
                        AF.Derivative_Erf,
                        bias=bs_sb[:, u * M + m : u * M + m + 1],
                    )
                return e_s

            def s_el(u, e_s, q_bf):
                dens = sp.tile([128, BL], f32, name="den", tag="den")
                tree_reduce(e_s, 8, dens)
                cs = sp.tile([128, BL], f32, name="crec", tag="crec")
                nc.vector.reciprocal_approx_fast(out=cs[:], in_=dens[:])

                q_v = (
                    q_bf[:, u * BL : (u + 1) * BL]
                    .unsqueeze(1)
                    .broadcast_to([128, 7, BL])
                )
                cs_v = cs.unsqueeze(1).broadcast_to([128, 7, BL])
                wp = wpp.tile([128, 7 * BL], bf16, name="wp", tag="wp")
                wp_v = wp.rearrange("p (m b) -> p m b", m=7)
                es_v = e_s[:, BL:].rearrange("p (m b) -> p m b", m=7)
                hh_v = hhat[u].rearrange("p (m b) -> p m b", m=7)
                # wp = e_s * (1/dens)  (gate, unnormalized-e trick)
                nc.vector.tensor_tensor(wp_v, es_v, cs_v, op=ALU.mult)
                if t > 0:
                    tq = tp.tile([128, 7 * BL], bf16, name="ehtq", tag="ehtq")
                    tq_v = tq.rearrange("p (m b) -> p m b", m=7)
                    nc.vector.tensor_tensor(tq_v, q_v, hh_v, op=ALU.subtract)
                    nc.vector.tensor_mul(wp[:], wp[:], tq[:])  # v = s*(q-h)
                    nc.gpsimd.tensor_add(wp[:], hhat[u][:], wp[:])  # h + v
                else:
                    nc.vector.tensor_tensor(wp_v, wp_v, q_v, op=ALU.mult)
                for mi in range(7):
                    nc.vector.tensor_scalar_mul(
                        hhat[u][:, mi * BL : (mi + 1) * BL],
                        wp[:, mi * BL : (mi + 1) * BL],
                        float(DECAY[mi + 1]),
                    )
                tree_reduce(hhat[u], 7, h_f32[:, u * BL : (u + 1) * BL])

            e_tiles = {0: s_mm(0)}

            # ---------------- q phase ----------------
            q_bf = stp.tile([128, UT * BL], bf16, name="q_bf", bufs=1)
            for uq in range(UT):
                psq = pq.tile([128, BL], f32, name="ps_q", tag="ps_q")
                mm_group(psq, wq_sb, uq * KT * 128, xt_t, rh_bf if t > 0 else None, t > 0)
                nc.scalar.activation(
                    q_bf[:, uq * BL : (uq + 1) * BL], psq[:],
                    AF.Tanh, bias=bq_sb[:, uq : uq + 1],
                )

            for u in range(UT):
                if u + 1 < UT:
                    e_tiles[u + 1] = s_mm(u + 1)
                s_el(u, e_tiles.pop(u), q_bf)

            hT = stp.tile([128, UT * BL], bf16, name="hT")
            nc.scalar.copy(hT[:], h_f32[:])

            # ---------------- y phase (fp32 matmul for precision) ----------
            psy = py.tile([3, BL], f32, name="ps_y", tag="ps_y")
            for k in range(UT):
                nc.tensor.matmul(
                    psy[:], wo_sb[:, k * 3 : (k + 1) * 3],
                    h_f32[:, k * BL : (k + 1) * BL],
                    start=(k == 0), stop=(k == UT - 1),
                )
            y_sb = stp.tile([3, BL], f32, name="y_sb")
            nc.scalar.copy(y_sb[:], psy[:])
            nc.sync.dma_start(y_d[t], y_sb[:])

    nc.compile()
    _program_cache["nc"] = nc
    return nc


def _prep_shared(W_r, b_r, W_q, b_q, W_s, b_s, W_out):
    import ml_dtypes

    bf = ml_dtypes.bfloat16

    def perm_w(w):  # [768, 4096] -> [128, (m,u,k,c)]
        a = np.ascontiguousarray(w, np.float32).reshape(KT, 128, UT, 128, M)
        return np.ascontiguousarray(
            a.transpose(1, 4, 2, 0, 3).reshape(128, M * UT * KT * 128)
        ).astype(bf)

    wr = perm_w(W_r)
    ws = perm_w(W_s)
    wq = np.ascontiguousarray(
        np.asarray(W_q, np.float32)
        .reshape(KT, 128, UT, 128)
        .transpose(1, 2, 0, 3)
        .reshape(128, UT * KT * 128)
    ).astype(bf)
    wo = np.ascontiguousarray(
        np.asarray(W_out, np.float32).reshape(UT, 128, 3).transpose(1, 0, 2).reshape(128, UT * 3)
    )
    biasr = np.ascontiguousarray(
        (np.asarray(b_r, np.float32).reshape(UT, 128, M) - LN_TAU).transpose(1, 0, 2).reshape(128, UT * M)
    )
    biass = np.ascontiguousarray(
        (np.asarray(b_s, np.float32).reshape(UT, 128, M) - LN_TAU).transpose(1, 0, 2).reshape(128, UT * M)
    )
    biasq = np.ascontiguousarray(np.asarray(b_q, np.float32).reshape(UT, 128).T)
    return dict(wr=wr, ws=ws, wq=wq, wo=wo, biasr=biasr, biass=biass, biasq=biasq)


def kernel(x, W_r, b_r, W_q, b_q, W_s, b_s, W_out, b_out):
    _install_axon_hooks_shim()
    from concourse.bass_utils import run_bass_kernel_spmd

    import ml_dtypes

    bf = ml_dtypes.bfloat16

    nc = _build_program()
    shared = _prep_shared(W_r, b_r, W_q, b_q, W_s, b_s, W_out)

    x = np.asarray(x, np.float32)
    in_maps = []
    for c in range(N_CORES):
        xc = x[c * BL : (c + 1) * BL]  # [BL, T, F]
        # -> [t, partition(=f%128? no: f k-tile), k*BL + b]
        xt = np.ascontiguousarray(
            xc.transpose(1, 2, 0).reshape(T, 2, 128, BL).transpose(0, 2, 1, 3).reshape(T, 128, 2 * BL)
        ).astype(bf)
        in_maps.append({"xt": xt, **shared})

    try:
        res = run_bass_kernel_spmd(nc, in_maps, list(range(N_CORES)))
    except Exception:
        # device pool may be wedged from an earlier crash — reset and retry
        try:
            lib = ctypes.CDLL("/opt/axon/libaxon_pjrt.so")
            lib.axon_reset.restype = ctypes.c_int64
            lib.axon_reset()
        except OSError:
            pass
        res = run_bass_kernel_spmd(nc, in_maps, list(range(N_CORES)))
    _program_cache["last_result"] = res

    out = np.empty((B, T, 3), np.float32)
    for c in range(N_CORES):
        y = res.results[c]["y"]  # [T, 3, BL]
        out[c * BL : (c + 1) * BL] = y.transpose(2, 0, 1)
    return out + np.asarray(b_out, np.float32)



# revision 7
# speedup vs baseline: 1.0921x; 1.0921x over previous
"""CTGRU forward kernel for 8 trn2 NeuronCores (data-parallel over batch).

Layout on device (per core, local batch BL=512):
  - All per-step tensors live as [U_partitions, m*BL + b] ("layout C", m-major
    free dim), so the hidden state h comes out of the m-reduction already in
    the [U, B] orientation the next step's matmuls need as their moving
    operand -> zero transposes inside the recurrence.
  - softmax(-(z - LN_TAU)^2) is computed as Derivative_Erf(z + (b - LN_TAU))
    = (2/sqrt(pi)) * exp(-d^2); the constant cancels in the normalization.
  - DECAY[0] == 0 exactly, so h_hat[..., m=0] is identically zero: the state
    stores only m=1..7 (7 slices) and all elementwise work skips m=0.
  - All DVE elementwise traffic is bf16 (2x mode); DECAY is folded into the
    per-m gate scale tile csd = DECAY_m/den so the state update collapses to
    one scalar_tensor_tensor per m: h' = h*DECAY_m + (e*csd)*(q-h).
"""

import contextlib
import ctypes
import sys
import types

import numpy as np

B, T, F, U, M = 4096, 16, 256, 512, 8
N_CORES = 8
BL = B // N_CORES  # 512
KT = (F + U) // 128  # 6 K-tiles over the fused dim
UT = U // 128  # 4 u-tiles

LN_TAU = (np.arange(M, dtype=np.float32) * (0.5 * np.log(10.0))).astype(np.float32)
DECAY = np.exp(-0.04 / (LN_TAU + 1e-7)).astype(np.float32)  # DECAY[0] == 0.0


def _install_axon_hooks_shim():
    """Make `antenv.axon_hooks` importable when the image lacks it, so
    BASS_TRACE-triggered profiling in run_bass_kernel_spmd can't crash us."""
    name = "antenv.axon_hooks"
    if name in sys.modules:
        return
    so_path = "/opt/axon/libaxon_pjrt.so"

    def _build_hook():
        try:
            lib = ctypes.CDLL(so_path)
        except OSError:
            return None
        if not hasattr(lib, "axon_start_nrt_profile"):
            return None
        lib.axon_start_nrt_profile.argtypes = [
            ctypes.POINTER(ctypes.c_int64),
            ctypes.c_size_t,
        ]
        lib.axon_start_nrt_profile.restype = ctypes.c_int64
        lib.axon_stop_nrt_profile.argtypes = [ctypes.c_char_p]
        lib.axon_stop_nrt_profile.restype = ctypes.c_int64

        @contextlib.contextmanager
        def _hook(output_dir, device_ids):
            import jax

            jax.devices()
            if device_ids:
                ids = (ctypes.c_int64 * len(device_ids))(*device_ids)
                rc = lib.axon_start_nrt_profile(ids, len(device_ids))
            else:
                rc = lib.axon_start_nrt_profile(None, 0)
            if rc != 0:
                raise RuntimeError(f"axon_start_nrt_profile rc={rc}")
            try:
                yield
            finally:
                n = lib.axon_stop_nrt_profile(str(output_dir).encode())
                print(f"profile: {n} file(s) written to {output_dir}", file=sys.stderr)

        return _hook

    mod = types.ModuleType(name)
    holder = [_build_hook()]
    mod.get_axon_ntff_profile_hook = lambda: holder[0]
    mod.set_axon_ntff_profile_hook = lambda h: holder.__setitem__(0, h)
    sys.modules[name] = mod
    try:
        import antenv

        antenv.axon_hooks = mod
    except ImportError:
        pass


_program_cache = {}


def _build_program():
    if "nc" in _program_cache:
        return _program_cache["nc"]

    import concourse.bass as bass
    import concourse.tile as tile
    from concourse import bacc, mybir

    f32 = mybir.dt.float32
    bf16 = mybir.dt.bfloat16
    AF = mybir.ActivationFunctionType
    ALU = mybir.AluOpType

    nc = bacc.Bacc("TRN2", target_bir_lowering=False, debug=False)

    xt_d = nc.dram_tensor("xt", [T, 128, 2 * BL], bf16, kind="ExternalInput").ap()
    wr_d = nc.dram_tensor("wr", [128, M * UT * KT * 128], bf16, kind="ExternalInput").ap()
    ws_d = nc.dram_tensor("ws", [128, M * UT * KT * 128], bf16, kind="ExternalInput").ap()
    wq_d = nc.dram_tensor("wq", [128, UT * KT * 128], bf16, kind="ExternalInput").ap()
    wo_d = nc.dram_tensor("wo", [128, UT * 3], bf16, kind="ExternalInput").ap()
    br_d = nc.dram_tensor("biasr", [128, UT * M], f32, kind="ExternalInput").ap()
    bs_d = nc.dram_tensor("biass", [128, UT * M], f32, kind="ExternalInput").ap()
    bq_d = nc.dram_tensor("biasq", [128, UT], f32, kind="ExternalInput").ap()
    y_d = nc.dram_tensor("y", [T, 3, BL], f32, kind="ExternalOutput").ap()

    with tile.TileContext(nc) as tc, contextlib.ExitStack() as ctx:
        const = ctx.enter_context(tc.tile_pool(name="const", bufs=1))
        state = ctx.enter_context(tc.tile_pool(name="state", bufs=1))
        wsp = ctx.enter_context(tc.tile_pool(name="wsp", bufs=2))
        rtp = ctx.enter_context(tc.tile_pool(name="rtp", bufs=1))
        xp = ctx.enter_context(tc.tile_pool(name="xp", bufs=2))
        ep = ctx.enter_context(tc.tile_pool(name="ep", bufs=2))
        tp = ctx.enter_context(tc.tile_pool(name="tp", bufs=2))
        wpp = ctx.enter_context(tc.tile_pool(name="wpp", bufs=1))
        sp = ctx.enter_context(tc.tile_pool(name="sp", bufs=2))
        stp = ctx.enter_context(tc.tile_pool(name="stp", bufs=2))
        htp = ctx.enter_context(tc.tile_pool(name="htp", bufs=2))
        pmm = ctx.enter_context(tc.tile_pool(name="pmm", bufs=5, space="PSUM"))
        pq = ctx.enter_context(tc.tile_pool(name="pq", bufs=2, space="PSUM"))
        py = ctx.enter_context(tc.tile_pool(name="py", bufs=1, space="PSUM"))

        # ---- weight / bias preload ----
        wr_sb = const.tile([128, M * UT * KT * 128], bf16, name="wr_sb")
        for m in range(M):
            sl = slice(m * UT * KT * 128, (m + 1) * UT * KT * 128)
            nc.sync.dma_start(wr_sb[:, sl], wr_d[:, sl])
        wq_sb = const.tile([128, UT * KT * 128], bf16, name="wq_sb")
        nc.sync.dma_start(wq_sb[:], wq_d[:])
        wo_sb = const.tile([128, UT * 3], bf16, name="wo_sb")
        nc.sync.dma_start(wo_sb[:], wo_d[:])
        br_sb = const.tile([128, UT * M], f32, name="br_sb")
        nc.sync.dma_start(br_sb[:], br_d[:])
        bs_sb = const.tile([128, UT * M], f32, name="bs_sb")
        nc.sync.dma_start(bs_sb[:], bs_d[:])
        bq_sb = const.tile([128, UT], f32, name="bq_sb")
        nc.sync.dma_start(bq_sb[:], bq_d[:])

        # DECAY pattern tile: slice mi holds DECAY[mi+1] (m=1..7)
        decpat = const.tile([128, 7 * BL], bf16, name="decpat")
        for mi in range(7):
            nc.vector.memset(decpat[:, mi * BL : (mi + 1) * BL], float(DECAY[mi + 1]))

        # h_hat state: m-slices 1..7 only (slice 0 is identically zero)
        hhat = [
            state.tile([128, 7 * BL], bf16, name=f"hhat{u}", tag=f"hhat{u}")
            for u in range(UT)
        ]

        hts = None  # list of 4 [128, BL] bf16 tiles: h(t-1) per u-tile

        def tree8(src, out):
            """out[128,BL] = sum of 8 contiguous BL-slices of src (bf16 tree)."""
            t1 = rtp.tile([128, 4 * BL], bf16, name="rt1", tag="rt1")
            t2 = rtp.tile([128, 2 * BL], bf16, name="rt2", tag="rt2")
            nc.vector.tensor_add(t1[:], src[:, : 4 * BL], src[:, 4 * BL :])
            nc.vector.tensor_add(t2[:], t1[:, : 2 * BL], t1[:, 2 * BL :])
            nc.vector.tensor_add(out[:], t2[:, :BL], t2[:, BL:])

        def tree7(src, out):
            """out[128,BL] = sum of 7 contiguous BL-slices of src (bf16 tree)."""
            t1 = rtp.tile([128, 4 * BL], bf16, name="rt1", tag="rt1")
            t2 = rtp.tile([128, 2 * BL], bf16, name="rt2", tag="rt2")
            nc.vector.tensor_add(t1[:, : 3 * BL], src[:, : 3 * BL], src[:, 4 * BL : 7 * BL])
            nc.vector.tensor_add(t2[:, :BL], t1[:, :BL], t1[:, BL : 2 * BL])
            nc.vector.tensor_add(
                t2[:, BL : 2 * BL], t1[:, 2 * BL : 3 * BL], src[:, 3 * BL : 4 * BL]
            )
            nc.vector.tensor_add(out[:], t2[:, :BL], t2[:, BL : 2 * BL])

        def mm_group(ps, w_sb, base, rhs, n_k):
            """Accumulate the fused matmul into psum `ps`. rhs: list of K-tile
            moving operands (x k-tiles, then h k-tiles)."""
            for k in range(n_k):
                nc.tensor.matmul(
                    ps[:],
                    w_sb[:, base + k * 128 : base + (k + 1) * 128],
                    rhs[k],
                    start=(k == 0), stop=(k == n_k - 1),
                )

        def emit_y(t, h_tiles):
            psy = py.tile([3, BL], f32, name="ps_y", tag="ps_y")
            for k in range(UT):
                nc.tensor.matmul(
                    psy[:], wo_sb[:, k * 3 : (k + 1) * 3], h_tiles[k][:],
                    start=(k == 0), stop=(k == UT - 1),
                )
            y_sb = stp.tile([3, BL], f32, name="y_sb")
            nc.scalar.copy(y_sb[:], psy[:])
            nc.sync.dma_start(y_d[t], y_sb[:])

        for t in range(T):
            xt_t = xp.tile([128, 2 * BL], bf16, name="xt_t")
            nc.sync.dma_start(xt_t[:], xt_d[t])
            rhs_x = [xt_t[:, 0:BL], xt_t[:, BL : 2 * BL]]
            rhs_full = rhs_x + [h[:] for h in hts] if t > 0 else None

            # ---------------- r phase (t=0: h_hat==0 makes r irrelevant) ----
            if t > 0:
                rh_bf = stp.tile([128, UT * BL], bf16, name="rh_bf", bufs=1)
                e_rs = []
                for u in range(UT):
                    e_r = ep.tile([128, M * BL], bf16, name="e_t", tag="e_t")
                    for m in range(M):
                        ps = pmm.tile([128, BL], f32, name="ps_mm", tag="ps_mm")
                        base = (m * UT + u) * KT * 128
                        mm_group(ps, wr_sb, base, rhs_full, KT)
                        nc.scalar.activation(
                            e_r[:, m * BL : (m + 1) * BL], ps[:],
                            AF.Derivative_Erf,
                            bias=br_sb[:, u * M + m : u * M + m + 1],
                        )
                    if u == 0:
                        # PE: y-phase of the previous step (hT ready, fills
                        # the pipeline right after the first r-group).
                        emit_y(t - 1, hts)
                    # gpsimd: e*h products (consumed by the eh tree below)
                    eh = tp.tile([128, 7 * BL], bf16, name="ehtq", tag="ehtq", bufs=2)
                    nc.gpsimd.tensor_mul(eh[:], e_r[:, BL:], hhat[u][:])
                    # DVE: softmax denominator + reciprocal
                    denr = sp.tile([128, BL], f32, name="den", tag="den", bufs=1)
                    tree8(e_r, denr)
                    cr = sp.tile([128, BL], f32, name="crec", tag="crec", bufs=4)
                    nc.vector.reciprocal_approx_fast(out=cr[:], in_=denr[:])
                    e_rs.append((eh, cr))
                for u in range(UT):
                    eh, cr = e_rs[u]
                    rhn = sp.tile([128, BL], bf16, name="rhn", tag="rhn", bufs=1)
                    tree7(eh, rhn)
                    nc.vector.tensor_mul(
                        rh_bf[:, u * BL : (u + 1) * BL], rhn[:], cr[:]
                    )

            # ------- s phase matmuls / elementwise, interleaved with q -----
            hts_new = [
                htp.tile([128, BL], bf16, name=f"hT{u}", tag=f"hT{u}")
                for u in range(UT)
            ]

            def s_mm(u):
                e_s = ep.tile([128, M * BL], bf16, name="e_t", tag="e_t")
                for m in range(M):
                    wsch = wsp.tile([128, KT * 128], bf16, name="wsch")
                    base = (m * UT + u) * KT * 128
                    nc.sync.dma_start(wsch[:], ws_d[:, base : base + KT * 128])
                    ps = pmm.tile([128, BL], f32, name="ps_mm", tag="ps_mm")
                    rhs = rhs_full if t > 0 else rhs_x
                    mm_group(ps, wsch, 0, rhs, KT if t > 0 else 2)
                    nc.scalar.activation(
                        e_s[:, m * BL : (m + 1) * BL], ps[:],
                        AF.Derivative_Erf,
                        bias=bs_sb[:, u * M + m : u * M + m + 1],
                    )
                return e_s

            def s_el(u, e_s, q_bf, tq):
                dens = sp.tile([128, BL], f32, name="den", tag="den", bufs=1)
                tree8(e_s, dens)
                cs = sp.tile([128, BL], f32, name="crec", tag="crec", bufs=4)
                nc.vector.reciprocal_approx_fast(out=cs[:], in_=dens[:])
                cs_bf = sp.tile([128, BL], bf16, name="csbf", tag="csbf", bufs=1)
                nc.vector.tensor_scalar_mul(cs_bf[:], cs[:], 1.0)
                # csd[m] = DECAY_m / den  (gate scale incl. decay)
                csd = tp.tile([128, 7 * BL], bf16, name="csd", tag="csd")
                csd_v = csd.rearrange("p (m b) -> p m b", m=7)
                cs_v = cs_bf.unsqueeze(1).broadcast_to([128, 7, BL])
                dec_v = decpat.rearrange("p (m b) -> p m b", m=7)
                nc.vector.tensor_tensor(csd_v, cs_v, dec_v, op=ALU.mult)
                # g = e * csd  (= DECAY_m * s_m)
                g = wpp.tile([128, 7 * BL], bf16, name="wp", tag="wp")
                nc.vector.tensor_mul(g[:], e_s[:, BL:], csd[:])
                if t > 0:
                    nc.vector.tensor_mul(g[:], g[:], tq[:])  # g = D*s*(q-h)
                    # h' = h*DECAY_m + g  (one fused op per m-slice)
                    for mi in range(7):
                        sl = slice(mi * BL, (mi + 1) * BL)
                        nc.vector.scalar_tensor_tensor(
                            hhat[u][:, sl], hhat[u][:, sl],
                            float(DECAY[mi + 1]), g[:, sl],
                            op0=ALU.mult, op1=ALU.add,
                        )
                else:
                    q_v = (
                        q_bf[:, u * BL : (u + 1) * BL]
                        .unsqueeze(1)
                        .broadcast_to([128, 7, BL])
                    )
                    g_v = g.rearrange("p (m b) -> p m b", m=7)
                    hh_v = hhat[u].rearrange("p (m b) -> p m b", m=7)
                    nc.vector.tensor_tensor(hh_v, g_v, q_v, op=ALU.mult)
                tree7(hhat[u], hts_new[u])

            e_tiles = {0: s_mm(0)}

            # ---------------- q phase ----------------
            q_bf = stp.tile([128, UT * BL], bf16, name="q_bf", bufs=1)
            for uq in range(UT):
                psq = pq.tile([128, BL], f32, name="ps_q", tag="ps_q")
                if t > 0:
                    rhs_q = rhs_x + [
                        rh_bf[:, k * BL : (k + 1) * BL] for k in range(UT)
                    ]
                    mm_group(psq, wq_sb, uq * KT * 128, rhs_q, KT)
                else:
                    mm_group(psq, wq_sb, uq * KT * 128, rhs_x, 2)
                nc.scalar.activation(
                    q_bf[:, uq * BL : (uq + 1) * BL], psq[:],
                    AF.Tanh, bias=bq_sb[:, uq : uq + 1],
                )

            for u in range(UT):
                if u + 1 < UT:
                    e_tiles[u + 1] = s_mm(u + 1)
                tq = None
                if t > 0:
                    # gpsimd: q - h (broadcast q over the 7 m-slices)
                    tq = tp.tile([128, 7 * BL], bf16, name="ehtq", tag="ehtq", bufs=2)
                    tq_v = tq.rearrange("p (m b) -> p m b", m=7)
                    q_v = (
                        q_bf[:, u * BL : (u + 1) * BL]
                        .unsqueeze(1)
                        .broadcast_to([128, 7, BL])
                    )
                    hh_v = hhat[u].rearrange("p (m b) -> p m b", m=7)
                    nc.gpsimd.tensor_tensor(tq_v, q_v, hh_v, op=ALU.subtract)
                s_el(u, e_tiles.pop(u), q_bf, tq)

            hts = hts_new

        emit_y(T - 1, hts)

    nc.compile()
    _program_cache["nc"] = nc
    return nc


def _prep_shared(W_r, b_r, W_q, b_q, W_s, b_s, W_out):
    import ml_dtypes

    bf = ml_dtypes.bfloat16

    def perm_w(w):  # [768, 4096] -> [128, (m,u,k,c)]
        a = np.ascontiguousarray(w, np.float32).reshape(KT, 128, UT, 128, M)
        return np.ascontiguousarray(
            a.transpose(1, 4, 2, 0, 3).reshape(128, M * UT * KT * 128)
        ).astype(bf)

    wr = perm_w(W_r)
    ws = perm_w(W_s)
    wq = np.ascontiguousarray(
        np.asarray(W_q, np.float32)
        .reshape(KT, 128, UT, 128)
        .transpose(1, 2, 0, 3)
        .reshape(128, UT * KT * 128)
    ).astype(bf)
    wo = np.ascontiguousarray(
        np.asarray(W_out, np.float32).reshape(UT, 128, 3).transpose(1, 0, 2).reshape(128, UT * 3)
    ).astype(bf)
    biasr = np.ascontiguousarray(
        (np.asarray(b_r, np.float32).reshape(UT, 128, M) - LN_TAU).transpose(1, 0, 2).reshape(128, UT * M)
    )
    biass = np.ascontiguousarray(
        (np.asarray(b_s, np.float32).reshape(UT, 128, M) - LN_TAU).transpose(1, 0, 2).reshape(128, UT * M)
    )
    biasq = np.ascontiguousarray(np.asarray(b_q, np.float32).reshape(UT, 128).T)
    return dict(wr=wr, ws=ws, wq=wq, wo=wo, biasr=biasr, biass=biass, biasq=biasq)


def kernel(x, W_r, b_r, W_q, b_q, W_s, b_s, W_out, b_out):
    _install_axon_hooks_shim()
    from concourse.bass_utils import run_bass_kernel_spmd

    import ml_dtypes

    bf = ml_dtypes.bfloat16

    nc = _build_program()
    shared = _prep_shared(W_r, b_r, W_q, b_q, W_s, b_s, W_out)

    x = np.asarray(x, np.float32)
    in_maps = []
    for c in range(N_CORES):
        xc = x[c * BL : (c + 1) * BL]  # [BL, T, F]
        xt = np.ascontiguousarray(
            xc.transpose(1, 2, 0).reshape(T, 2, 128, BL).transpose(0, 2, 1, 3).reshape(T, 128, 2 * BL)
        ).astype(bf)
        in_maps.append({"xt": xt, **shared})

    try:
        res = run_bass_kernel_spmd(nc, in_maps, list(range(N_CORES)))
    except Exception:
        # device pool may be wedged from an earlier crash — reset and retry
        try:
            lib = ctypes.CDLL("/opt/axon/libaxon_pjrt.so")
            lib.axon_reset.restype = ctypes.c_int64
            lib.axon_reset()
        except OSError:
            pass
        res = run_bass_kernel_spmd(nc, in_maps, list(range(N_CORES)))
    _program_cache["last_result"] = res

    out = np.empty((B, T, 3), np.float32)
    for c in range(N_CORES):
        y = res.results[c]["y"]  # [T, 3, BL]
        out[c * BL : (c + 1) * BL] = y.transpose(2, 0, 1)
    return out + np.asarray(b_out, np.float32)


# revision 8
# speedup vs baseline: 1.2434x; 1.1385x over previous
"""CTGRU forward kernel for 8 trn2 NeuronCores (data-parallel over batch).

Layout on device (per core, local batch BL=512):
  - All per-step tensors live as [U_partitions, m*BL + b] ("layout C", m-major
    free dim), so the hidden state h comes out of the m-reduction already in
    the [U, B] orientation the next step's matmuls need as their moving
    operand -> zero transposes inside the recurrence.
  - softmax(-(z - LN_TAU)^2) is computed as Derivative_Erf(z + (b - LN_TAU))
    = (2/sqrt(pi)) * exp(-d^2); the constant cancels in the normalization.
  - DECAY[0] == 0 exactly, so h_hat[..., m=0] is identically zero: the state
    stores only m=1..7 (7 slices) and all elementwise work skips m=0.
  - All DVE elementwise traffic is bf16 (2x mode); DECAY is folded into the
    per-m gate scale tile csd = DECAY_m/den so the state update collapses to
    one scalar_tensor_tensor per m: h' = h*DECAY_m + (e*csd)*(q-h).
"""

import contextlib
import ctypes
import sys
import types

import numpy as np

B, T, F, U, M = 4096, 16, 256, 512, 8
N_CORES = 8
BL = B // N_CORES  # 512
KT = (F + U) // 128  # 6 K-tiles over the fused dim
UT = U // 128  # 4 u-tiles

LN_TAU = (np.arange(M, dtype=np.float32) * (0.5 * np.log(10.0))).astype(np.float32)
DECAY = np.exp(-0.04 / (LN_TAU + 1e-7)).astype(np.float32)  # DECAY[0] == 0.0


def _install_axon_hooks_shim():
    """Make `antenv.axon_hooks` importable when the image lacks it, so
    BASS_TRACE-triggered profiling in run_bass_kernel_spmd can't crash us."""
    name = "antenv.axon_hooks"
    if name in sys.modules:
        return
    so_path = "/opt/axon/libaxon_pjrt.so"

    def _build_hook():
        try:
            lib = ctypes.CDLL(so_path)
        except OSError:
            return None
        if not hasattr(lib, "axon_start_nrt_profile"):
            return None
        lib.axon_start_nrt_profile.argtypes = [
            ctypes.POINTER(ctypes.c_int64),
            ctypes.c_size_t,
        ]
        lib.axon_start_nrt_profile.restype = ctypes.c_int64
        lib.axon_stop_nrt_profile.argtypes = [ctypes.c_char_p]
        lib.axon_stop_nrt_profile.restype = ctypes.c_int64

        @contextlib.contextmanager
        def _hook(output_dir, device_ids):
            import jax

            jax.devices()
            if device_ids:
                ids = (ctypes.c_int64 * len(device_ids))(*device_ids)
                rc = lib.axon_start_nrt_profile(ids, len(device_ids))
            else:
                rc = lib.axon_start_nrt_profile(None, 0)
            if rc != 0:
                raise RuntimeError(f"axon_start_nrt_profile rc={rc}")
            try:
                yield
            finally:
                n = lib.axon_stop_nrt_profile(str(output_dir).encode())
                print(f"profile: {n} file(s) written to {output_dir}", file=sys.stderr)

        return _hook

    mod = types.ModuleType(name)
    holder = [_build_hook()]
    mod.get_axon_ntff_profile_hook = lambda: holder[0]
    mod.set_axon_ntff_profile_hook = lambda h: holder.__setitem__(0, h)
    sys.modules[name] = mod
    try:
        import antenv

        antenv.axon_hooks = mod
    except ImportError:
        pass


_program_cache = {}


def _build_program():
    if "nc" in _program_cache:
        return _program_cache["nc"]

    import concourse.bass as bass
    import concourse.tile as tile
    from concourse import bacc, mybir

    f32 = mybir.dt.float32
    bf16 = mybir.dt.bfloat16
    AF = mybir.ActivationFunctionType
    ALU = mybir.AluOpType

    nc = bacc.Bacc("TRN2", target_bir_lowering=False, debug=False)

    xt_d = nc.dram_tensor("xt", [T, 128, 2 * BL], bf16, kind="ExternalInput").ap()
    wr_d = nc.dram_tensor("wr", [128, M * UT * KT * 128], bf16, kind="ExternalInput").ap()
    ws_d = nc.dram_tensor("ws", [128, M * UT * KT * 128], bf16, kind="ExternalInput").ap()
    wq_d = nc.dram_tensor("wq", [128, UT * KT * 128], bf16, kind="ExternalInput").ap()
    wo_d = nc.dram_tensor("wo", [128, UT * 3], bf16, kind="ExternalInput").ap()
    br_d = nc.dram_tensor("biasr", [128, UT * M], f32, kind="ExternalInput").ap()
    bs_d = nc.dram_tensor("biass", [128, UT * M], f32, kind="ExternalInput").ap()
    bq_d = nc.dram_tensor("biasq", [128, UT], f32, kind="ExternalInput").ap()
    y_d = nc.dram_tensor("y", [T, 3, BL], f32, kind="ExternalOutput").ap()

    with tile.TileContext(nc) as tc, contextlib.ExitStack() as ctx:
        const = ctx.enter_context(tc.tile_pool(name="const", bufs=1))
        state = ctx.enter_context(tc.tile_pool(name="state", bufs=1))
        wsp = ctx.enter_context(tc.tile_pool(name="wsp", bufs=2))
        rtp = ctx.enter_context(tc.tile_pool(name="rtp", bufs=1))
        xp = ctx.enter_context(tc.tile_pool(name="xp", bufs=2))
        ep = ctx.enter_context(tc.tile_pool(name="ep", bufs=2))
        tp = ctx.enter_context(tc.tile_pool(name="tp", bufs=2))
        wpp = ctx.enter_context(tc.tile_pool(name="wpp", bufs=1))
        sp = ctx.enter_context(tc.tile_pool(name="sp", bufs=2))
        stp = ctx.enter_context(tc.tile_pool(name="stp", bufs=2))
        htp = ctx.enter_context(tc.tile_pool(name="htp", bufs=2))
        pmm = ctx.enter_context(tc.tile_pool(name="pmm", bufs=5, space="PSUM"))
        pq = ctx.enter_context(tc.tile_pool(name="pq", bufs=2, space="PSUM"))
        py = ctx.enter_context(tc.tile_pool(name="py", bufs=1, space="PSUM"))

        # ---- weight / bias preload ----
        wr_sb = const.tile([128, M * UT * KT * 128], bf16, name="wr_sb")
        for m in range(M):
            sl = slice(m * UT * KT * 128, (m + 1) * UT * KT * 128)
            nc.sync.dma_start(wr_sb[:, sl], wr_d[:, sl])
        wq_sb = const.tile([128, UT * KT * 128], bf16, name="wq_sb")
        nc.sync.dma_start(wq_sb[:], wq_d[:])
        wo_sb = const.tile([128, UT * 3], bf16, name="wo_sb")
        nc.sync.dma_start(wo_sb[:], wo_d[:])
        br_sb = const.tile([128, UT * M], f32, name="br_sb")
        nc.sync.dma_start(br_sb[:], br_d[:])
        bs_sb = const.tile([128, UT * M], f32, name="bs_sb")
        nc.sync.dma_start(bs_sb[:], bs_d[:])
        bq_sb = const.tile([128, UT], f32, name="bq_sb")
        nc.sync.dma_start(bq_sb[:], bq_d[:])

        # DECAY pattern tile: slice mi holds DECAY[mi+1] (m=1..7)
        decpat = const.tile([128, 7 * BL], bf16, name="decpat")
        for mi in range(7):
            nc.vector.memset(decpat[:, mi * BL : (mi + 1) * BL], float(DECAY[mi + 1]))

        # h_hat state: m-slices 1..7 only (slice 0 is identically zero)
        hhat = [
            state.tile([128, 7 * BL], bf16, name=f"hhat{u}", tag=f"hhat{u}")
            for u in range(UT)
        ]

        hts = None  # list of 4 [128, BL] bf16 tiles: h(t-1) per u-tile

        def tree8(src, out):
            """out[128,BL] = sum of 8 contiguous BL-slices of src (bf16 tree)."""
            t1 = rtp.tile([128, 4 * BL], bf16, name="rt1", tag="rt1")
            t2 = rtp.tile([128, 2 * BL], bf16, name="rt2", tag="rt2")
            nc.vector.tensor_add(t1[:], src[:, : 4 * BL], src[:, 4 * BL :])
            nc.vector.tensor_add(t2[:], t1[:, : 2 * BL], t1[:, 2 * BL :])
            nc.vector.tensor_add(out[:], t2[:, :BL], t2[:, BL:])

        def tree7(src, out):
            """out[128,BL] = sum of 7 contiguous BL-slices of src (bf16 tree)."""
            t1 = rtp.tile([128, 4 * BL], bf16, name="rt1", tag="rt1")
            t2 = rtp.tile([128, 2 * BL], bf16, name="rt2", tag="rt2")
            nc.vector.tensor_add(t1[:, : 3 * BL], src[:, : 3 * BL], src[:, 4 * BL : 7 * BL])
            nc.vector.tensor_add(t2[:, :BL], t1[:, :BL], t1[:, BL : 2 * BL])
            nc.vector.tensor_add(
                t2[:, BL : 2 * BL], t1[:, 2 * BL : 3 * BL], src[:, 3 * BL : 4 * BL]
            )
            nc.vector.tensor_add(out[:], t2[:, :BL], t2[:, BL : 2 * BL])

        def mm_group(ps, w_sb, base, rhs, n_k):
            """Accumulate the fused matmul into psum `ps`. rhs: list of K-tile
            moving operands (x k-tiles, then h k-tiles)."""
            for k in range(n_k):
                nc.tensor.matmul(
                    ps[:],
                    w_sb[:, base + k * 128 : base + (k + 1) * 128],
                    rhs[k],
                    start=(k == 0), stop=(k == n_k - 1),
                )

        def emit_y(t, h_tiles):
            psy = py.tile([3, BL], f32, name="ps_y", tag="ps_y")
            for k in range(UT):
                nc.tensor.matmul(
                    psy[:], wo_sb[:, k * 3 : (k + 1) * 3], h_tiles[k][:],
                    start=(k == 0), stop=(k == UT - 1),
                )
            y_sb = stp.tile([3, BL], f32, name="y_sb")
            nc.scalar.copy(y_sb[:], psy[:])
            nc.sync.dma_start(y_d[t], y_sb[:])

        for t in range(T):
            xt_t = xp.tile([128, 2 * BL], bf16, name="xt_t")
            nc.sync.dma_start(xt_t[:], xt_d[t])
            rhs_x = [xt_t[:, 0:BL], xt_t[:, BL : 2 * BL]]
            rhs_full = rhs_x + [h[:] for h in hts] if t > 0 else None

            # ---------------- r phase (t=0: h_hat==0 makes r irrelevant) ----
            if t > 0:
                rh_bf = stp.tile([128, UT * BL], bf16, name="rh_bf", bufs=1)
                for u in range(UT):
                    e_r = ep.tile([128, M * BL], bf16, name="e_t", tag="e_t")
                    for m in range(M):
                        ps = pmm.tile([128, BL], f32, name="ps_mm", tag="ps_mm")
                        base = (m * UT + u) * KT * 128
                        mm_group(ps, wr_sb, base, rhs_full, KT)
                        nc.scalar.activation(
                            e_r[:, m * BL : (m + 1) * BL], ps[:],
                            AF.Derivative_Erf,
                            bias=br_sb[:, u * M + m : u * M + m + 1],
                        )
                    if u == 0:
                        # PE: y-phase of the previous step (hT ready, fills
                        # the pipeline right after the first r-group).
                        emit_y(t - 1, hts)
                    # DVE: e*h products, denominator tree, weighted sum
                    eh = tp.tile([128, 7 * BL], bf16, name="ehtq", tag="ehtq", bufs=2)
                    nc.vector.tensor_mul(eh[:], e_r[:, BL:], hhat[u][:])
                    denr = sp.tile([128, BL], f32, name="den", tag="den", bufs=1)
                    tree8(e_r, denr)
                    cr = sp.tile([128, BL], f32, name="crec", tag="crec", bufs=2)
                    nc.vector.reciprocal_approx_fast(out=cr[:], in_=denr[:])
                    rhn = sp.tile([128, BL], bf16, name="rhn", tag="rhn", bufs=1)
                    tree7(eh, rhn)
                    nc.vector.tensor_mul(
                        rh_bf[:, u * BL : (u + 1) * BL], rhn[:], cr[:]
                    )

            # ------- s phase matmuls / elementwise, interleaved with q -----
            hts_new = [
                htp.tile([128, BL], bf16, name=f"hT{u}", tag=f"hT{u}")
                for u in range(UT)
            ]

            def s_mm(u):
                e_s = ep.tile([128, M * BL], bf16, name="e_t", tag="e_t")
                for m in range(M):
                    wsch = wsp.tile([128, KT * 128], bf16, name="wsch")
                    base = (m * UT + u) * KT * 128
                    nc.sync.dma_start(wsch[:], ws_d[:, base : base + KT * 128])
                    ps = pmm.tile([128, BL], f32, name="ps_mm", tag="ps_mm")
                    rhs = rhs_full if t > 0 else rhs_x
                    mm_group(ps, wsch, 0, rhs, KT if t > 0 else 2)
                    nc.scalar.activation(
                        e_s[:, m * BL : (m + 1) * BL], ps[:],
                        AF.Derivative_Erf,
                        bias=bs_sb[:, u * M + m : u * M + m + 1],
                    )
                return e_s

            def s_el(u, e_s, q_bf, tq):
                dens = sp.tile([128, BL], f32, name="den", tag="den", bufs=1)
                tree8(e_s, dens)
                cs = sp.tile([128, BL], f32, name="crec", tag="crec", bufs=2)
                nc.vector.reciprocal_approx_fast(out=cs[:], in_=dens[:])
                cs_bf = sp.tile([128, BL], bf16, name="csbf", tag="csbf", bufs=1)
                nc.vector.tensor_scalar_mul(cs_bf[:], cs[:], 1.0)
                # s8 = e / den  (softmax gate, broadcast 1/den over m)
                s8 = wpp.tile([128, 7 * BL], bf16, name="wp", tag="wp")
                s8_v = s8.rearrange("p (m b) -> p m b", m=7)
                cs_v = cs_bf.unsqueeze(1).broadcast_to([128, 7, BL])
                es_v = e_s[:, BL:].rearrange("p (m b) -> p m b", m=7)
                nc.vector.tensor_tensor(s8_v, es_v, cs_v, op=ALU.mult)
                if t > 0:
                    nc.vector.tensor_mul(s8[:], s8[:], tq[:])  # s*(q-h)
                    nc.vector.tensor_add(tq[:], hhat[u][:], s8[:])  # h + s(q-h)
                    nc.vector.tensor_mul(hhat[u][:], tq[:], decpat[:])
                else:
                    q_v = (
                        q_bf[:, u * BL : (u + 1) * BL]
                        .unsqueeze(1)
                        .broadcast_to([128, 7, BL])
                    )
                    nc.vector.tensor_tensor(s8_v, s8_v, q_v, op=ALU.mult)
                    nc.vector.tensor_mul(hhat[u][:], s8[:], decpat[:])
                tree7(hhat[u], hts_new[u])

            e_tiles = {0: s_mm(0)}

            # ---------------- q phase ----------------
            q_bf = stp.tile([128, UT * BL], bf16, name="q_bf", bufs=1)
            for uq in range(UT):
                psq = pq.tile([128, BL], f32, name="ps_q", tag="ps_q")
                if t > 0:
                    rhs_q = rhs_x + [
                        rh_bf[:, k * BL : (k + 1) * BL] for k in range(UT)
                    ]
                    mm_group(psq, wq_sb, uq * KT * 128, rhs_q, KT)
                else:
                    mm_group(psq, wq_sb, uq * KT * 128, rhs_x, 2)
                nc.scalar.activation(
                    q_bf[:, uq * BL : (uq + 1) * BL], psq[:],
                    AF.Tanh, bias=bq_sb[:, uq : uq + 1],
                )

            for u in range(UT):
                if u + 1 < UT:
                    e_tiles[u + 1] = s_mm(u + 1)
                tq = None
                if t > 0:
                    # DVE: q - h (broadcast q over the 7 m-slices)
                    tq = tp.tile([128, 7 * BL], bf16, name="ehtq", tag="ehtq", bufs=2)
                    tq_v = tq.rearrange("p (m b) -> p m b", m=7)
                    q_v = (
                        q_bf[:, u * BL : (u + 1) * BL]
                        .unsqueeze(1)
                        .broadcast_to([128, 7, BL])
                    )
                    hh_v = hhat[u].rearrange("p (m b) -> p m b", m=7)
                    nc.vector.tensor_tensor(tq_v, q_v, hh_v, op=ALU.subtract)
                s_el(u, e_tiles.pop(u), q_bf, tq)

            hts = hts_new

        emit_y(T - 1, hts)

    nc.compile()
    _program_cache["nc"] = nc
    return nc


def _prep_shared(W_r, b_r, W_q, b_q, W_s, b_s, W_out):
    import ml_dtypes

    bf = ml_dtypes.bfloat16

    def perm_w(w):  # [768, 4096] -> [128, (m,u,k,c)]
        a = np.ascontiguousarray(w, np.float32).reshape(KT, 128, UT, 128, M)
        return np.ascontiguousarray(
            a.transpose(1, 4, 2, 0, 3).reshape(128, M * UT * KT * 128)
        ).astype(bf)

    wr = perm_w(W_r)
    ws = perm_w(W_s)
    wq = np.ascontiguousarray(
        np.asarray(W_q, np.float32)
        .reshape(KT, 128, UT, 128)
        .transpose(1, 2, 0, 3)
        .reshape(128, UT * KT * 128)
    ).astype(bf)
    wo = np.ascontiguousarray(
        np.asarray(W_out, np.float32).reshape(UT, 128, 3).transpose(1, 0, 2).reshape(128, UT * 3)
    ).astype(bf)
    biasr = np.ascontiguousarray(
        (np.asarray(b_r, np.float32).reshape(UT, 128, M) - LN_TAU).transpose(1, 0, 2).reshape(128, UT * M)
    )
    biass = np.ascontiguousarray(
        (np.asarray(b_s, np.float32).reshape(UT, 128, M) - LN_TAU).transpose(1, 0, 2).reshape(128, UT * M)
    )
    biasq = np.ascontiguousarray(np.asarray(b_q, np.float32).reshape(UT, 128).T)
    return dict(wr=wr, ws=ws, wq=wq, wo=wo, biasr=biasr, biass=biass, biasq=biasq)


def kernel(x, W_r, b_r, W_q, b_q, W_s, b_s, W_out, b_out):
    _install_axon_hooks_shim()
    from concourse.bass_utils import run_bass_kernel_spmd

    import ml_dtypes

    bf = ml_dtypes.bfloat16

    nc = _build_program()
    shared = _prep_shared(W_r, b_r, W_q, b_q, W_s, b_s, W_out)

    x = np.asarray(x, np.float32)
    in_maps = []
    for c in range(N_CORES):
        xc = x[c * BL : (c + 1) * BL]  # [BL, T, F]
        xt = np.ascontiguousarray(
            xc.transpose(1, 2, 0).reshape(T, 2, 128, BL).transpose(0, 2, 1, 3).reshape(T, 128, 2 * BL)
        ).astype(bf)
        in_maps.append({"xt": xt, **shared})

    try:
        res = run_bass_kernel_spmd(nc, in_maps, list(range(N_CORES)))
    except Exception:
        # device pool may be wedged from an earlier crash — reset and retry
        try:
            lib = ctypes.CDLL("/opt/axon/libaxon_pjrt.so")
            lib.axon_reset.restype = ctypes.c_int64
            lib.axon_reset()
        except OSError:
            pass
        res = run_bass_kernel_spmd(nc, in_maps, list(range(N_CORES)))
    _program_cache["last_result"] = res

    out = np.empty((B, T, 3), np.float32)
    for c in range(N_CORES):
        y = res.results[c]["y"]  # [T, 3, BL]
        out[c * BL : (c + 1) * BL] = y.transpose(2, 0, 1)
    return out + np.asarray(b_out, np.float32)


# revision 9
# speedup vs baseline: 1.5327x; 1.2327x over previous
"""CTGRU forward kernel for 8 trn2 NeuronCores (data-parallel over batch).

Layout on device (per core, local batch BL=512):
  - All per-step tensors live as [U_partitions, m*BL + b] ("layout C", m-major
    free dim), so the hidden state h comes out of the m-reduction already in
    the [U, B] orientation the next step's matmuls need as their moving
    operand -> zero transposes inside the recurrence.
  - softmax(-(z - LN_TAU)^2) is computed as Derivative_Erf(z + (b - LN_TAU))
    = (2/sqrt(pi)) * exp(-d^2); the constant cancels in the normalization.
  - DECAY[0] == 0 exactly, so h_hat[..., m=0] is identically zero: the state
    stores only m=1..7 (7 slices) and all elementwise work skips m=0.
  - All DVE elementwise traffic is bf16 (2x mode); DECAY is folded into the
    per-m gate scale tile csd = DECAY_m/den so the state update collapses to
    one scalar_tensor_tensor per m: h' = h*DECAY_m + (e*csd)*(q-h).
"""

import contextlib
import ctypes
import sys
import types

import numpy as np

B, T, F, U, M = 4096, 16, 256, 512, 8
N_CORES = 8
BL = B // N_CORES  # 512
KT = (F + U) // 128  # 6 K-tiles over the fused dim
UT = U // 128  # 4 u-tiles

LN_TAU = (np.arange(M, dtype=np.float32) * (0.5 * np.log(10.0))).astype(np.float32)
DECAY = np.exp(-0.04 / (LN_TAU + 1e-7)).astype(np.float32)  # DECAY[0] == 0.0


def _install_axon_hooks_shim():
    """Make `antenv.axon_hooks` importable when the image lacks it, so
    BASS_TRACE-triggered profiling in run_bass_kernel_spmd can't crash us."""
    name = "antenv.axon_hooks"
    if name in sys.modules:
        return
    so_path = "/opt/axon/libaxon_pjrt.so"

    def _build_hook():
        try:
            lib = ctypes.CDLL(so_path)
        except OSError:
            return None
        if not hasattr(lib, "axon_start_nrt_profile"):
            return None
        lib.axon_start_nrt_profile.argtypes = [
            ctypes.POINTER(ctypes.c_int64),
            ctypes.c_size_t,
        ]
        lib.axon_start_nrt_profile.restype = ctypes.c_int64
        lib.axon_stop_nrt_profile.argtypes = [ctypes.c_char_p]
        lib.axon_stop_nrt_profile.restype = ctypes.c_int64

        @contextlib.contextmanager
        def _hook(output_dir, device_ids):
            import jax

            jax.devices()
            if device_ids:
                ids = (ctypes.c_int64 * len(device_ids))(*device_ids)
                rc = lib.axon_start_nrt_profile(ids, len(device_ids))
            else:
                rc = lib.axon_start_nrt_profile(None, 0)
            if rc != 0:
                raise RuntimeError(f"axon_start_nrt_profile rc={rc}")
            try:
                yield
            finally:
                n = lib.axon_stop_nrt_profile(str(output_dir).encode())
                print(f"profile: {n} file(s) written to {output_dir}", file=sys.stderr)

        return _hook

    mod = types.ModuleType(name)
    holder = [_build_hook()]
    mod.get_axon_ntff_profile_hook = lambda: holder[0]
    mod.set_axon_ntff_profile_hook = lambda h: holder.__setitem__(0, h)
    sys.modules[name] = mod
    try:
        import antenv

        antenv.axon_hooks = mod
    except ImportError:
        pass


_program_cache = {}


def _build_program():
    if "nc" in _program_cache:
        return _program_cache["nc"]

    import concourse.bass as bass
    import concourse.tile as tile
    from concourse import bacc, mybir

    f32 = mybir.dt.float32
    bf16 = mybir.dt.bfloat16
    AF = mybir.ActivationFunctionType
    ALU = mybir.AluOpType

    nc = bacc.Bacc("TRN2", target_bir_lowering=False, debug=False)

    f8 = mybir.dt.float8e4
    xt_d = nc.dram_tensor("xt", [T, 128, 2 * BL], bf16, kind="ExternalInput").ap()
    xt8_d = nc.dram_tensor("xt8", [T, 128, 2 * BL], f8, kind="ExternalInput").ap()
    id_d = nc.dram_tensor("ident", [128, 128], bf16, kind="ExternalInput").ap()
    wr_d = nc.dram_tensor("wr", [128, M * UT * KT * 128], f8, kind="ExternalInput").ap()
    ws_d = nc.dram_tensor("ws", [128, M * UT * KT * 128], bf16, kind="ExternalInput").ap()
    wq_d = nc.dram_tensor("wq", [128, UT * KT * 128], bf16, kind="ExternalInput").ap()
    wo_d = nc.dram_tensor("wo", [128, UT * 3], bf16, kind="ExternalInput").ap()
    br_d = nc.dram_tensor("biasr", [128, UT * M], f32, kind="ExternalInput").ap()
    bs_d = nc.dram_tensor("biass", [128, UT * M], f32, kind="ExternalInput").ap()
    bq_d = nc.dram_tensor("biasq", [128, UT], f32, kind="ExternalInput").ap()
    y_d = nc.dram_tensor("y", [T, 3, BL], f32, kind="ExternalOutput").ap()

    with tile.TileContext(nc) as tc, contextlib.ExitStack() as ctx:
        const = ctx.enter_context(tc.tile_pool(name="const", bufs=1))
        state = ctx.enter_context(tc.tile_pool(name="state", bufs=1))
        wsp = ctx.enter_context(tc.tile_pool(name="wsp", bufs=2))
        rtp = ctx.enter_context(tc.tile_pool(name="rtp", bufs=1))
        xp = ctx.enter_context(tc.tile_pool(name="xp", bufs=2))
        ep = ctx.enter_context(tc.tile_pool(name="ep", bufs=2))
        tp = ctx.enter_context(tc.tile_pool(name="tp", bufs=2))
        wpp = ctx.enter_context(tc.tile_pool(name="wpp", bufs=1))
        sp = ctx.enter_context(tc.tile_pool(name="sp", bufs=2))
        stp = ctx.enter_context(tc.tile_pool(name="stp", bufs=2))
        htp = ctx.enter_context(tc.tile_pool(name="htp", bufs=2))
        pmm = ctx.enter_context(tc.tile_pool(name="pmm", bufs=4, space="PSUM"))
        pq = ctx.enter_context(tc.tile_pool(name="pq", bufs=1, space="PSUM"))
        py = ctx.enter_context(tc.tile_pool(name="py", bufs=1, space="PSUM"))
        pden = ctx.enter_context(tc.tile_pool(name="pden", bufs=2, space="PSUM"))

        # ---- weight / bias preload ----
        wr_sb = const.tile([128, M * UT * KT * 128], f8, name="wr_sb")
        for m in range(M):
            sl = slice(m * UT * KT * 128, (m + 1) * UT * KT * 128)
            nc.sync.dma_start(wr_sb[:, sl], wr_d[:, sl])
        wq_sb = const.tile([128, UT * KT * 128], bf16, name="wq_sb")
        nc.sync.dma_start(wq_sb[:], wq_d[:])
        id_sb = const.tile([128, 128], bf16, name="id_sb")
        nc.sync.dma_start(id_sb[:], id_d[:])
        wo_sb = const.tile([128, UT * 3], bf16, name="wo_sb")
        nc.sync.dma_start(wo_sb[:], wo_d[:])
        br_sb = const.tile([128, UT * M], f32, name="br_sb")
        nc.sync.dma_start(br_sb[:], br_d[:])
        bs_sb = const.tile([128, UT * M], f32, name="bs_sb")
        nc.sync.dma_start(bs_sb[:], bs_d[:])
        bq_sb = const.tile([128, UT], f32, name="bq_sb")
        nc.sync.dma_start(bq_sb[:], bq_d[:])

        # DECAY pattern tile: slice mi holds DECAY[mi+1] (m=1..7)
        decpat = const.tile([128, 7 * BL], bf16, name="decpat")
        for mi in range(7):
            nc.vector.memset(decpat[:, mi * BL : (mi + 1) * BL], float(DECAY[mi + 1]))

        # h_hat state: m-slices 1..7 only (slice 0 is identically zero)
        hhat = [
            state.tile([128, 7 * BL], bf16, name=f"hhat{u}", tag=f"hhat{u}")
            for u in range(UT)
        ]

        hts = None  # list of 4 [128, BL] bf16 tiles: h(t-1) per u-tile
        hT8 = None  # [128, UT*BL] fp8 copy of h(t-1) for the r-phase matmuls

        def tree8(src, out):
            """out[128,BL] = sum of 8 contiguous BL-slices of src (bf16 tree)."""
            t1 = rtp.tile([128, 4 * BL], bf16, name="rt1", tag="rt1")
            t2 = rtp.tile([128, 2 * BL], bf16, name="rt2", tag="rt2")
            nc.vector.tensor_add(t1[:], src[:, : 4 * BL], src[:, 4 * BL :])
            nc.vector.tensor_add(t2[:], t1[:, : 2 * BL], t1[:, 2 * BL :])
            nc.vector.tensor_add(out[:], t2[:, :BL], t2[:, BL:])

        def tree7(src, out):
            """out[128,BL] = sum of 7 contiguous BL-slices of src (bf16 tree)."""
            t1 = rtp.tile([128, 4 * BL], bf16, name="rt1", tag="rt1")
            t2 = rtp.tile([128, 2 * BL], bf16, name="rt2", tag="rt2")
            nc.vector.tensor_add(t1[:, : 3 * BL], src[:, : 3 * BL], src[:, 4 * BL : 7 * BL])
            nc.vector.tensor_add(t2[:, :BL], t1[:, :BL], t1[:, BL : 2 * BL])
            nc.vector.tensor_add(
                t2[:, BL : 2 * BL], t1[:, 2 * BL : 3 * BL], src[:, 3 * BL : 4 * BL]
            )
            nc.vector.tensor_add(out[:], t2[:, :BL], t2[:, BL : 2 * BL])

        def mm_group_r8(ps, base, xt8_t, hT8):
            """r-phase fused matmul in fp8 DoubleRow: 3 instrs cover K=768."""
            pairs = [
                xt8_t.rearrange("p (two b) -> p two b", two=2),
                hT8[:, : 2 * BL].rearrange("p (two b) -> p two b", two=2),
                hT8[:, 2 * BL :].rearrange("p (two b) -> p two b", two=2),
            ]
            for j in range(3):
                lhsT = wr_sb[:, base + j * 256 : base + (j + 1) * 256].rearrange(
                    "p (two c) -> p two c", two=2
                )
                nc.tensor.matmul(
                    ps[:], lhsT, pairs[j],
                    start=(j == 0), stop=(j == 2),
                    perf_mode=mybir.MatmulPerfMode.DoubleRow,
                )

        def den8_pe(e_tile):
            """den[128,BL] psum = sum of 8 BL-slices of e via identity matmuls."""
            dps = pden.tile([128, BL], f32, name="ps_den", tag="ps_den")
            for m in range(M):
                nc.tensor.matmul(
                    dps[:], id_sb[:], e_tile[:, m * BL : (m + 1) * BL],
                    start=(m == 0), stop=(m == M - 1),
                )
            den_sb = sp.tile([128, BL], f32, name="den", tag="den", bufs=2)
            nc.scalar.copy(den_sb[:], dps[:])
            return den_sb

        def mm_group(ps, w_sb, base, rhs, n_k):
            """Accumulate the fused matmul into psum `ps`. rhs: list of K-tile
            moving operands (x k-tiles, then h k-tiles)."""
            for k in range(n_k):
                nc.tensor.matmul(
                    ps[:],
                    w_sb[:, base + k * 128 : base + (k + 1) * 128],
                    rhs[k],
                    start=(k == 0), stop=(k == n_k - 1),
                )

        def emit_y(t, h_tiles):
            psy = py.tile([3, BL], f32, name="ps_y", tag="ps_y")
            for k in range(UT):
                nc.tensor.matmul(
                    psy[:], wo_sb[:, k * 3 : (k + 1) * 3], h_tiles[k][:],
                    start=(k == 0), stop=(k == UT - 1),
                )
            y_sb = stp.tile([3, BL], f32, name="y_sb")
            nc.scalar.copy(y_sb[:], psy[:])
            nc.sync.dma_start(y_d[t], y_sb[:])

        for t in range(T):
            xt_t = xp.tile([128, 2 * BL], bf16, name="xt_t")
            nc.sync.dma_start(xt_t[:], xt_d[t])
            xt8_t = xp.tile([128, 2 * BL], f8, name="xt8_t")
            nc.sync.dma_start(xt8_t[:], xt8_d[t])
            rhs_x = [xt_t[:, 0:BL], xt_t[:, BL : 2 * BL]]
            rhs_full = rhs_x + [h[:] for h in hts] if t > 0 else None

            def den_fin(pe):
                # PE denominator + DVE reciprocal + weighted-sum normalize
                u, e_r, rhn = pe
                den_sb = den8_pe(e_r)
                cr = sp.tile([128, BL], f32, name="crec", tag="crec", bufs=2)
                nc.vector.reciprocal_approx_fast(out=cr[:], in_=den_sb[:])
                nc.vector.tensor_mul(
                    rh_bf[:, u * BL : (u + 1) * BL], rhn[:], cr[:]
                )

            # ---------------- r phase (t=0: h_hat==0 makes r irrelevant) ----
            if t > 0:
                rh_bf = stp.tile([128, UT * BL], bf16, name="rh_bf", bufs=1)
                prev_er = None
                for u in range(UT):
                    e_r = ep.tile([128, M * BL], bf16, name="e_t", tag="e_t")
                    for m in range(M):
                        ps = pmm.tile([128, BL], f32, name="ps_mm", tag="ps_mm")
                        base = (m * UT + u) * KT * 128
                        mm_group_r8(ps, base, xt8_t, hT8)
                        nc.scalar.activation(
                            e_r[:, m * BL : (m + 1) * BL], ps[:],
                            AF.Derivative_Erf, scale=0.125,
                            bias=br_sb[:, u * M + m : u * M + m + 1],
                        )
                    if u == 0:
                        # PE: y-phase of the previous step (hT ready, fills
                        # the pipeline right after the first r-group).
                        emit_y(t - 1, hts)
                    if prev_er is not None:
                        den_fin(prev_er)
                    # DVE: e*h products + weighted-sum tree
                    eh = tp.tile([128, 7 * BL], bf16, name="ehtq", tag="ehtq", bufs=2)
                    nc.vector.tensor_mul(eh[:], e_r[:, BL:], hhat[u][:])
                    rhn = sp.tile([128, BL], bf16, name="rhn", tag="rhn", bufs=1)
                    tree7(eh, rhn)
                    prev_er = (u, e_r, rhn)
                den_fin(prev_er)

            # ------- s phase matmuls / elementwise, interleaved with q -----
            hts_new = [
                htp.tile([128, BL], bf16, name=f"hT{u}", tag=f"hT{u}")
                for u in range(UT)
            ]
            hT8_new = htp.tile([128, UT * BL], f8, name="hT8", tag="hT8")

            def s_mm(u):
                e_s = ep.tile([128, M * BL], bf16, name="e_t", tag="e_t")
                for m in range(M):
                    wsch = wsp.tile([128, KT * 128], bf16, name="wsch")
                    base = (m * UT + u) * KT * 128
                    nc.sync.dma_start(wsch[:], ws_d[:, base : base + KT * 128])
                    ps = pmm.tile([128, BL], f32, name="ps_mm", tag="ps_mm")
                    rhs = rhs_full if t > 0 else rhs_x
                    mm_group(ps, wsch, 0, rhs, KT if t > 0 else 2)
                    nc.scalar.activation(
                        e_s[:, m * BL : (m + 1) * BL], ps[:],
                        AF.Derivative_Erf,
                        bias=bs_sb[:, u * M + m : u * M + m + 1],
                    )
                return e_s

            def s_el(u, e_s, q_bf, tq):
                dens = den8_pe(e_s)
                cs = sp.tile([128, BL], f32, name="crec", tag="crec", bufs=2)
                nc.vector.reciprocal_approx_fast(out=cs[:], in_=dens[:])
                cs_bf = sp.tile([128, BL], bf16, name="csbf", tag="csbf", bufs=1)
                nc.vector.tensor_scalar_mul(cs_bf[:], cs[:], 1.0)
                # s8 = e / den  (softmax gate, broadcast 1/den over m)
                s8 = wpp.tile([128, 7 * BL], bf16, name="wp", tag="wp")
                s8_v = s8.rearrange("p (m b) -> p m b", m=7)
                cs_v = cs_bf.unsqueeze(1).broadcast_to([128, 7, BL])
                es_v = e_s[:, BL:].rearrange("p (m b) -> p m b", m=7)
                nc.vector.tensor_tensor(s8_v, es_v, cs_v, op=ALU.mult)
                if t > 0:
                    nc.vector.tensor_mul(s8[:], s8[:], tq[:])  # s*(q-h)
                    nc.vector.tensor_add(tq[:], hhat[u][:], s8[:])  # h + s(q-h)
                    nc.vector.tensor_mul(hhat[u][:], tq[:], decpat[:])
                else:
                    q_v = (
                        q_bf[:, u * BL : (u + 1) * BL]
                        .unsqueeze(1)
                        .broadcast_to([128, 7, BL])
                    )
                    nc.vector.tensor_tensor(s8_v, s8_v, q_v, op=ALU.mult)
                    nc.vector.tensor_mul(hhat[u][:], s8[:], decpat[:])
                tree7(hhat[u], hts_new[u])
                nc.vector.tensor_scalar_mul(
                    hT8_new[:, u * BL : (u + 1) * BL], hts_new[u][:], 1.0
                )

            e_tiles = {0: s_mm(0)}

            # ---------------- q phase ----------------
            q_bf = stp.tile([128, UT * BL], bf16, name="q_bf", bufs=1)
            for uq in range(UT):
                psq = pq.tile([128, BL], f32, name="ps_q", tag="ps_q")
                if t > 0:
                    rhs_q = rhs_x + [
                        rh_bf[:, k * BL : (k + 1) * BL] for k in range(UT)
                    ]
                    mm_group(psq, wq_sb, uq * KT * 128, rhs_q, KT)
                else:
                    mm_group(psq, wq_sb, uq * KT * 128, rhs_x, 2)
                nc.scalar.activation(
                    q_bf[:, uq * BL : (uq + 1) * BL], psq[:],
                    AF.Tanh, bias=bq_sb[:, uq : uq + 1],
                )

            for u in range(UT):
                if u + 1 < UT:
                    e_tiles[u + 1] = s_mm(u + 1)
                tq = None
                if t > 0:
                    # DVE: q - h (broadcast q over the 7 m-slices)
                    tq = tp.tile([128, 7 * BL], bf16, name="ehtq", tag="ehtq", bufs=2)
                    tq_v = tq.rearrange("p (m b) -> p m b", m=7)
                    q_v = (
                        q_bf[:, u * BL : (u + 1) * BL]
                        .unsqueeze(1)
                        .broadcast_to([128, 7, BL])
                    )
                    hh_v = hhat[u].rearrange("p (m b) -> p m b", m=7)
                    nc.vector.tensor_tensor(tq_v, q_v, hh_v, op=ALU.subtract)
                s_el(u, e_tiles.pop(u), q_bf, tq)

            hts = hts_new
            hT8 = hT8_new

        emit_y(T - 1, hts)

    nc.compile()
    _program_cache["nc"] = nc
    return nc


def _prep_shared(W_r, b_r, W_q, b_q, W_s, b_s, W_out):
    import ml_dtypes

    bf = ml_dtypes.bfloat16

    def perm_w(w):  # [768, 4096] -> [128, (m,u,k,c)]
        a = np.ascontiguousarray(w, np.float32).reshape(KT, 128, UT, 128, M)
        return np.ascontiguousarray(
            a.transpose(1, 4, 2, 0, 3).reshape(128, M * UT * KT * 128)
        ).astype(bf)

    from concourse import mybir

    f8np = mybir.dt.np(mybir.dt.float8e4)
    a = np.ascontiguousarray(W_r, np.float32).reshape(KT, 128, UT, 128, M)
    wr = np.ascontiguousarray(
        (a * 8.0).transpose(1, 4, 2, 0, 3).reshape(128, M * UT * KT * 128)
    ).astype(f8np)
    ws = perm_w(W_s)
    wq = np.ascontiguousarray(
        np.asarray(W_q, np.float32)
        .reshape(KT, 128, UT, 128)
        .transpose(1, 2, 0, 3)
        .reshape(128, UT * KT * 128)
    ).astype(bf)
    wo = np.ascontiguousarray(
        np.asarray(W_out, np.float32).reshape(UT, 128, 3).transpose(1, 0, 2).reshape(128, UT * 3)
    ).astype(bf)
    biasr = np.ascontiguousarray(
        (np.asarray(b_r, np.float32).reshape(UT, 128, M) - LN_TAU).transpose(1, 0, 2).reshape(128, UT * M)
    )
    biass = np.ascontiguousarray(
        (np.asarray(b_s, np.float32).reshape(UT, 128, M) - LN_TAU).transpose(1, 0, 2).reshape(128, UT * M)
    )
    biasq = np.ascontiguousarray(np.asarray(b_q, np.float32).reshape(UT, 128).T)
    ident = np.eye(128, dtype=np.float32).astype(bf)
    return dict(wr=wr, ws=ws, wq=wq, wo=wo, biasr=biasr, biass=biass, biasq=biasq,
                ident=ident, _f8np=f8np)


def kernel(x, W_r, b_r, W_q, b_q, W_s, b_s, W_out, b_out):
    _install_axon_hooks_shim()
    from concourse.bass_utils import run_bass_kernel_spmd

    import ml_dtypes

    bf = ml_dtypes.bfloat16

    nc = _build_program()
    shared = _prep_shared(W_r, b_r, W_q, b_q, W_s, b_s, W_out)

    x = np.asarray(x, np.float32)
    in_maps = []
    for c in range(N_CORES):
        xc = x[c * BL : (c + 1) * BL]  # [BL, T, F]
        xtf = np.ascontiguousarray(
            xc.transpose(1, 2, 0).reshape(T, 2, 128, BL).transpose(0, 2, 1, 3).reshape(T, 128, 2 * BL)
        )
        xt = xtf.astype(bf)
        xt8 = xtf.astype(shared["_f8np"])
        m = {k: v for k, v in shared.items() if k != "_f8np"}
        in_maps.append({"xt": xt, "xt8": xt8, **m})

    try:
        res = run_bass_kernel_spmd(nc, in_maps, list(range(N_CORES)))
    except Exception:
        # device pool may be wedged from an earlier crash — reset and retry
        try:
            lib = ctypes.CDLL("/opt/axon/libaxon_pjrt.so")
            lib.axon_reset.restype = ctypes.c_int64
            lib.axon_reset()
        except OSError:
            pass
        res = run_bass_kernel_spmd(nc, in_maps, list(range(N_CORES)))
    _program_cache["last_result"] = res

    out = np.empty((B, T, 3), np.float32)
    for c in range(N_CORES):
        y = res.results[c]["y"]  # [T, 3, BL]
        out[c * BL : (c + 1) * BL] = y.transpose(2, 0, 1)
    return out + np.asarray(b_out, np.float32)


# revision 29
# speedup vs baseline: 1.8851x; 1.2300x over previous
"""CTGRU forward kernel for 8 trn2 NeuronCores (data-parallel over batch).

Layout on device (per core, local batch BL=512):
  - All per-step tensors live as [U_partitions, m*BL + b] ("layout C", m-major
    free dim), so the hidden state h comes out of the m-reduction already in
    the [U, B] orientation the next step's matmuls need as their moving
    operand -> zero transposes inside the recurrence.
  - softmax(-(z - LN_TAU)^2) is computed as Derivative_Erf(z + (b - LN_TAU))
    = (2/sqrt(pi)) * exp(-d^2); the constant cancels in the normalization.
  - DECAY[0] == 0 exactly, so h_hat[..., m=0] is identically zero: the state
    stores only m=1..7 (7 slices) and all elementwise work skips m=0.
  - All DVE elementwise traffic is bf16 (2x mode); DECAY is folded into the
    per-m gate scale tile csd = DECAY_m/den so the state update collapses to
    one scalar_tensor_tensor per m: h' = h*DECAY_m + (e*csd)*(q-h).
"""

import contextlib
import ctypes
import sys
import types

import numpy as np

B, T, F, U, M = 4096, 16, 256, 512, 8
N_CORES = 8
BL = B // N_CORES  # 512
KT = (F + U) // 128  # 6 K-tiles over the fused dim
UT = U // 128  # 4 u-tiles

LN_TAU = (np.arange(M, dtype=np.float32) * (0.5 * np.log(10.0))).astype(np.float32)
DECAY = np.exp(-0.04 / (LN_TAU + 1e-7)).astype(np.float32)  # DECAY[0] == 0.0


def _install_axon_hooks_shim():
    """Make `antenv.axon_hooks` importable when the image lacks it, so
    BASS_TRACE-triggered profiling in run_bass_kernel_spmd can't crash us."""
    name = "antenv.axon_hooks"
    if name in sys.modules:
        return
    so_path = "/opt/axon/libaxon_pjrt.so"

    def _build_hook():
        try:
            lib = ctypes.CDLL(so_path)
        except OSError:
            return None
        if not hasattr(lib, "axon_start_nrt_profile"):
            return None
        lib.axon_start_nrt_profile.argtypes = [
            ctypes.POINTER(ctypes.c_int64),
            ctypes.c_size_t,
        ]
        lib.axon_start_nrt_profile.restype = ctypes.c_int64
        lib.axon_stop_nrt_profile.argtypes = [ctypes.c_char_p]
        lib.axon_stop_nrt_profile.restype = ctypes.c_int64

        @contextlib.contextmanager
        def _hook(output_dir, device_ids):
            import jax

            jax.devices()
            if device_ids:
                ids = (ctypes.c_int64 * len(device_ids))(*device_ids)
                rc = lib.axon_start_nrt_profile(ids, len(device_ids))
            else:
                rc = lib.axon_start_nrt_profile(None, 0)
            if rc != 0:
                raise RuntimeError(f"axon_start_nrt_profile rc={rc}")
            try:
                yield
            finally:
                n = lib.axon_stop_nrt_profile(str(output_dir).encode())
                print(f"profile: {n} file(s) written to {output_dir}", file=sys.stderr)

        return _hook

    mod = types.ModuleType(name)
    holder = [_build_hook()]
    mod.get_axon_ntff_profile_hook = lambda: holder[0]
    mod.set_axon_ntff_profile_hook = lambda h: holder.__setitem__(0, h)
    sys.modules[name] = mod
    try:
        import antenv

        antenv.axon_hooks = mod
    except ImportError:
        pass


_program_cache = {}


def _build_program():
    if "nc" in _program_cache:
        return _program_cache["nc"]

    import concourse.bass as bass
    import concourse.tile as tile
    from concourse import bacc, mybir

    f32 = mybir.dt.float32
    bf16 = mybir.dt.bfloat16
    AF = mybir.ActivationFunctionType
    ALU = mybir.AluOpType

    nc = bacc.Bacc("TRN2", target_bir_lowering=False, debug=False)

    f8 = mybir.dt.float8e4
    xt_d = nc.dram_tensor("xt", [T, 128, 2 * BL], bf16, kind="ExternalInput").ap()
    xt8_d = nc.dram_tensor("xt8", [T, 128, 2 * BL], f8, kind="ExternalInput").ap()
    id_d = nc.dram_tensor("ident", [128, 128], bf16, kind="ExternalInput").ap()
    wr_d = nc.dram_tensor("wr", [128, M * UT * KT * 128], f8, kind="ExternalInput").ap()
    ws_d = nc.dram_tensor("ws", [128, M * UT * KT * 128], bf16, kind="ExternalInput").ap()
    wq_d = nc.dram_tensor("wq", [128, UT * KT * 128], bf16, kind="ExternalInput").ap()
    wo_d = nc.dram_tensor("wo", [128, UT * 3], bf16, kind="ExternalInput").ap()
    br_d = nc.dram_tensor("biasr", [128, UT * M], f32, kind="ExternalInput").ap()
    bs_d = nc.dram_tensor("biass", [128, UT * M], f32, kind="ExternalInput").ap()
    bq_d = nc.dram_tensor("biasq", [128, UT], f32, kind="ExternalInput").ap()
    y_d = nc.dram_tensor("y", [T, 3, BL], f32, kind="ExternalOutput").ap()

    with tile.TileContext(nc) as tc, contextlib.ExitStack() as ctx:
        const = ctx.enter_context(tc.tile_pool(name="const", bufs=1))
        state = ctx.enter_context(tc.tile_pool(name="state", bufs=1))
        wsp = ctx.enter_context(tc.tile_pool(name="wsp", bufs=6))
        rtp = ctx.enter_context(tc.tile_pool(name="rtp", bufs=1))
        xp = ctx.enter_context(tc.tile_pool(name="xp", bufs=3))
        ep = ctx.enter_context(tc.tile_pool(name="ep", bufs=3))
        tp = ctx.enter_context(tc.tile_pool(name="tp", bufs=2))
        wpp = ctx.enter_context(tc.tile_pool(name="wpp", bufs=1))
        sp = ctx.enter_context(tc.tile_pool(name="sp", bufs=2))
        stp = ctx.enter_context(tc.tile_pool(name="stp", bufs=2))
        htp = ctx.enter_context(tc.tile_pool(name="htp", bufs=2))
        pmm = ctx.enter_context(tc.tile_pool(name="pmm", bufs=5, space="PSUM"))
        py = ctx.enter_context(tc.tile_pool(name="py", bufs=1, space="PSUM"))
        pden = ctx.enter_context(tc.tile_pool(name="pden", bufs=2, space="PSUM"))

        # ---- weight / bias preload ----
        wr_sb = const.tile([128, M * UT * KT * 128], f8, name="wr_sb")
        for m in range(M):
            sl = slice(m * UT * KT * 128, (m + 1) * UT * KT * 128)
            nc.sync.dma_start(wr_sb[:, sl], wr_d[:, sl])
        wq_sb = const.tile([128, UT * KT * 128], bf16, name="wq_sb")
        nc.sync.dma_start(wq_sb[:], wq_d[:])
        id_sb = const.tile([128, 128], bf16, name="id_sb")
        nc.sync.dma_start(id_sb[:], id_d[:])
        wo_sb = const.tile([128, UT * 3], bf16, name="wo_sb")
        nc.sync.dma_start(wo_sb[:], wo_d[:])
        br_sb = const.tile([128, UT * M], f32, name="br_sb")
        nc.sync.dma_start(br_sb[:], br_d[:])
        bs_sb = const.tile([128, UT * M], f32, name="bs_sb")
        nc.sync.dma_start(bs_sb[:], bs_d[:])
        bq_sb = const.tile([128, UT], f32, name="bq_sb")
        nc.sync.dma_start(bq_sb[:], bq_d[:])

        # DECAY pattern tile: slice mi holds DECAY[mi+1] (m=1..7)
        decpat = const.tile([128, 7 * BL], bf16, name="decpat")
        for mi in range(7):
            nc.vector.memset(decpat[:, mi * BL : (mi + 1) * BL], float(DECAY[mi + 1]))

        # h_hat state: m-slices 1..7 only (slice 0 is identically zero)
        hhat = [
            state.tile([128, 7 * BL], bf16, name=f"hhat{u}", tag=f"hhat{u}")
            for u in range(UT)
        ]

        hts = None  # list of 4 [128, BL] bf16 tiles: h(t-1) per u-tile
        hT8 = None  # [128, UT*BL] fp8 copy of h(t-1) for the r-phase matmuls

        def tree8(src, out):
            """out[128,BL] = sum of 8 contiguous BL-slices of src (bf16 tree)."""
            t1 = rtp.tile([128, 4 * BL], bf16, name="rt1", tag="rt1")
            t2 = rtp.tile([128, 2 * BL], bf16, name="rt2", tag="rt2")
            nc.vector.tensor_add(t1[:], src[:, : 4 * BL], src[:, 4 * BL :])
            nc.vector.tensor_add(t2[:], t1[:, : 2 * BL], t1[:, 2 * BL :])
            nc.vector.tensor_add(out[:], t2[:, :BL], t2[:, BL:])

        def tree7(src, out):
            """out[128,BL] = sum of 7 contiguous BL-slices of src (bf16 tree)."""
            t1 = rtp.tile([128, 4 * BL], bf16, name="rt1", tag="rt1")
            t2 = rtp.tile([128, 2 * BL], bf16, name="rt2", tag="rt2")
            nc.vector.tensor_add(t1[:, : 3 * BL], src[:, : 3 * BL], src[:, 4 * BL : 7 * BL])
            nc.vector.tensor_add(t2[:, :BL], t1[:, :BL], t1[:, BL : 2 * BL])
            nc.vector.tensor_add(
                t2[:, BL : 2 * BL], t1[:, 2 * BL : 3 * BL], src[:, 3 * BL : 4 * BL]
            )
            nc.vector.tensor_add(out[:], t2[:, :BL], t2[:, BL : 2 * BL])

        def mm_group_r8(ps, base, xt8_t, hT8, head=True, tail=True):
            """r-phase fused matmul in fp8 DoubleRow: 3 instrs cover K=768."""
            hT8a, hT8b = hT8
            pairs = [
                xt8_t.rearrange("p (two b) -> p two b", two=2),
                hT8a.rearrange("p (two b) -> p two b", two=2),
                hT8b.rearrange("p (two b) -> p two b", two=2),
            ]
            js = ([0, 1] if head else []) + ([2] if tail else [])
            for j in js:
                lhsT = wr_sb[:, base + j * 256 : base + (j + 1) * 256].rearrange(
                    "p (two c) -> p two c", two=2
                )
                nc.tensor.matmul(
                    ps[:], lhsT, pairs[j],
                    start=(j == 0), stop=(j == 2),
                    perf_mode=mybir.MatmulPerfMode.DoubleRow,
                )

        def den_mm(dps, e_tile, m):
            """Accumulate e m-slice into the den psum via an identity matmul."""
            nc.tensor.matmul(
                dps[:], id_sb[:], e_tile[:, m * BL : (m + 1) * BL],
                start=(m == 0), stop=(m == M - 1),
            )

        def mm_group(ps, w_sb, base, rhs, n_k):
            """Accumulate the fused matmul into psum `ps`. rhs: list of K-tile
            moving operands (x k-tiles, then h k-tiles)."""
            for k in range(n_k):
                nc.tensor.matmul(
                    ps[:],
                    w_sb[:, base + k * 128 : base + (k + 1) * 128],
                    rhs[k],
                    start=(k == 0), stop=(k == n_k - 1),
                )

        def emit_y(t, h_tiles):
            psy = py.tile([3, BL], f32, name="ps_y", tag="ps_y")
            for k in range(UT):
                nc.tensor.matmul(
                    psy[:], wo_sb[:, k * 3 : (k + 1) * 3], h_tiles[k][:],
                    start=(k == 0), stop=(k == UT - 1),
                )
            y_sb = stp.tile([3, BL], f32, name="y_sb")
            nc.scalar.copy(y_sb[:], psy[:])
            nc.sync.dma_start(y_d[t], y_sb[:])

        pending_state = []

        def flush_state():
            while pending_state:
                u, etqd, cs_bf, dh0 = pending_state.pop()
                # per-m quanta: small enough that scheduler insertions into
                # the boundary-critical tail cost <0.4us each, not 1.9us
                with tc.high_priority(offset=-150):
                    for mi in range(7):
                        sl = slice(mi * BL, (mi + 1) * BL)
                        nc.vector.tensor_mul(etqd[:, sl], etqd[:, sl], cs_bf[:])
                        nc.vector.tensor_add(hhat[u][:, sl], dh0[:, sl], etqd[:, sl])

        for t in range(T):
            xt_t = xp.tile([128, 2 * BL], bf16, name="xt_t", tag="xt")
            nc.sync.dma_start(xt_t[:], xt_d[t])
            xt8_t = xp.tile([128, 2 * BL], f8, name="xt8_t", tag="xt8")
            nc.sync.dma_start(xt8_t[:], xt8_d[t])
            flush_state()
            rhs_x = [xt_t[:, 0:BL], xt_t[:, BL : 2 * BL]]
            rhs_full = rhs_x + [h[:] for h in hts] if t > 0 else None

                rh_bf = stp.tile([128, UT * BL], bf16, name="rh_bf", bufs=1)
                for u in range(UT):
                    e_r = ep.tile([128, M * BL], bf16, name="e_t", tag="e_t")
                    dps = pden.tile([128, BL], f32, name="ps_den", tag="ps_den")
                    pss = {}
                    if u == 0:
                        # run-ahead: x/h01 pairs of the first 4 groups fill
                        # the PE while u3's elementwise tail finishes h.
                        for m in range(4):
                            pss[m] = pmm.tile([128, BL], f32, name="ps_mm", tag="ps_mm")
                            base = (m * UT + u) * KT * 128
                            mm_group_r8(pss[m], base, xt8_t, hT8, tail=False)
                    for m in range(M):
                        base = (m * UT + u) * KT * 128
                        if m in pss:
                            ps = pss[m]
                            mm_group_r8(ps, base, xt8_t, hT8, head=False)
                        else:
                            ps = pmm.tile([128, BL], f32, name="ps_mm", tag="ps_mm")
                            mm_group_r8(ps, base, xt8_t, hT8)
                        nc.scalar.activation(
                            e_r[:, m * BL : (m + 1) * BL], ps[:],
                            AF.Derivative_Erf, scale=0.125,
                            bias=br_sb[:, u * M + m : u * M + m + 1],
                        )
                        if m >= 2:
                            den_mm(dps, e_r, m - 2)
                    if u == 0:
                        # PE: y-phase of the previous step (hT ready, fills
                        # the pipeline right after the first r-group).
                        emit_y(t - 1, hts)
                    den_mm(dps, e_r, M - 2)
                    den_mm(dps, e_r, M - 1)
                    # DVE: e*h products + weighted-sum tree + normalize
                    eh = tp.tile([128, 7 * BL], bf16, name="ehtq", tag="ehtq", bufs=2)
                    nc.vector.tensor_mul(eh[:], e_r[:, BL:], hhat[u][:])
                    rhn = sp.tile([128, BL], bf16, name="rhn", tag="rhn", bufs=1)
                    tree7(eh, rhn)
                    cr = sp.tile([128, BL], f32, name="crec", tag="crec", bufs=2)
                    nc.vector.reciprocal_approx_fast(out=cr[:], in_=dps[:])
                    nc.vector.tensor_mul(
                        rh_bf[:, u * BL : (u + 1) * BL], rhn[:], cr[:]
                    )            # ---------------- r phase (t=0: h_hat==0 makes r irrelevant) ----
            if t > 0:
                rh_bf = stp.tile([128, UT * BL], bf16, name="rh_bf", bufs=1)
                for u in range(UT):
                    e_r = ep.tile([128, M * BL], bf16, name="e_t", tag="e_t")
                    dps = pden.tile([128, BL], f32, name="ps_den", tag="ps_den")
                    pss = {}
                    if u == 0:
                        # run-ahead: x/h01 pairs of the first 4 groups fill
                        # the PE while u3's elementwise tail finishes h.
                        for m in range(4):
                            pss[m] = pmm.tile([128, BL], f32, name="ps_mm", tag="ps_mm")
                            base = (m * UT + u) * KT * 128
                            mm_group_r8(pss[m], base, xt8_t, hT8, tail=False)
                    for m in range(M):
                        base = (m * UT + u) * KT * 128
                        if m in pss:
                            ps = pss[m]
                            mm_group_r8(ps, base, xt8_t, hT8, head=False)
                        else:
                            ps = pmm.tile([128, BL], f32, name="ps_mm", tag="ps_mm")
                            mm_group_r8(ps, base, xt8_t, hT8)
                        nc.scalar.activation(
                            e_r[:, m * BL : (m + 1) * BL], ps[:],
                            AF.Derivative_Erf, scale=0.125,
                            bias=br_sb[:, u * M + m : u * M + m + 1],
                        )
                        if m >= 2:
                            den_mm(dps, e_r, m - 2)
                    if u == 0:
                        # PE: y-phase of the previous step (hT ready, fills
                        # the pipeline right after the first r-group).
                        emit_y(t - 1, hts)
                    den_mm(dps, e_r, M - 2)
                    den_mm(dps, e_r, M - 1)
                    # DVE: e*h products + weighted-sum tree + normalize
                    eh = tp.tile([128, 7 * BL], bf16, name="ehtq", tag="ehtq", bufs=2)
                    nc.vector.tensor_mul(eh[:], e_r[:, BL:], hhat[u][:])
                    rhn = sp.tile([128, BL], bf16, name="rhn", tag="rhn", bufs=1)
                    tree7(eh, rhn)
                    cr = sp.tile([128, BL], f32, name="crec", tag="crec", bufs=2)
                    nc.vector.reciprocal_approx_fast(out=cr[:], in_=dps[:])
                    nc.vector.tensor_mul(
                        rh_bf[:, u * BL : (u + 1) * BL], rhn[:], cr[:]
                    )

            # ------- s phase matmuls / elementwise, interleaved with q -----
            hts_new = [
                htp.tile([128, BL], bf16, name=f"hT{u}", tag=f"hT{u}")
                for u in range(UT)
            ]
            hT8a_new = htp.tile([128, 2 * BL], f8, name="hT8a", tag="hT8a")
            hT8b_new = htp.tile([128, 2 * BL], f8, name="hT8b", tag="hT8b")

            def s_mm(u, etqd=None, tqd=None):
                e_s = ep.tile([128, M * BL], bf16, name="e_t", tag="e_t")
                dps = pden.tile([128, BL], f32, name="ps_den", tag="ps_den")
                for m in range(M):
                    wsch = wsp.tile([128, KT * 128], bf16, name="wsch")
                    base = (m * UT + u) * KT * 128
                    nc.sync.dma_start(wsch[:], ws_d[:, base : base + KT * 128])
                    ps = pmm.tile([128, BL], f32, name="ps_mm", tag="ps_mm")
                    rhs = rhs_full if t > 0 else rhs_x
                    mm_group(ps, wsch, 0, rhs, KT if t > 0 else 2)
                    nc.scalar.activation(
                        e_s[:, m * BL : (m + 1) * BL], ps[:],
                        AF.Derivative_Erf,
                        bias=bs_sb[:, u * M + m : u * M + m + 1],
                    )
                    if m >= 2:
                        den_mm(dps, e_s, m - 2)
                    if etqd is not None and m >= 1:
                        sl = slice((m - 1) * BL, m * BL)
                        nc.vector.tensor_mul(
                            etqd[:, sl], e_s[:, m * BL : (m + 1) * BL], tqd[:, sl]
                        )
                        te1, te2 = te_tmp
                        if m == 3:
                            nc.vector.tensor_add(
                                te1[:, :BL], etqd[:, :BL], etqd[:, BL : 2 * BL]
                            )
                        elif m == 5:
                            nc.vector.tensor_add(
                                te1[:, BL : 2 * BL],
                                etqd[:, 2 * BL : 3 * BL], etqd[:, 3 * BL : 4 * BL],
                            )
                        elif m == 7:
                            nc.vector.tensor_add(
                                te1[:, 2 * BL : 3 * BL],
                                etqd[:, 4 * BL : 5 * BL], etqd[:, 5 * BL : 6 * BL],
                            )
                            nc.vector.tensor_add(
                                te2[:, :BL], te1[:, :BL], te1[:, BL : 2 * BL]
                            )
                den_mm(dps, e_s, M - 2)
                den_mm(dps, e_s, M - 1)
                return e_s, dps

            def s_el(u, e_s, dps, q_bf, tq, etqd=None, dh=None):
                cs = sp.tile([128, BL], f32, name="crec", tag="crec", bufs=2)
                nc.vector.reciprocal_approx_fast(out=cs[:], in_=dps[:])
                cs_bf = sp.tile([128, BL], bf16, name="csbf", tag="csbf", bufs=1)
                nc.vector.tensor_scalar_mul(cs_bf[:], cs[:], 1.0)
                cs_v = cs_bf.unsqueeze(1).broadcast_to([128, 7, BL])
                if etqd is not None:
                    # fast tail (last u): hT = tree(h*D) + tree(e*tq*D)/den;
                    # only 512-wide ops remain after the last activation.
                    te1, te2, tdh = dh[1]
                    nc.vector.tensor_add(
                        te2[:, BL : 2 * BL], te1[:, 2 * BL : 3 * BL], etqd[:, 6 * BL :]
                    )
                    teE = sp.tile([128, BL], bf16, name="teE", tag="teE", bufs=1)
                    nc.vector.tensor_add(teE[:], te2[:, :BL], te2[:, BL : 2 * BL])
                    sv = sp.tile([128, BL], bf16, name="sv", tag="sv", bufs=1)
                    nc.vector.tensor_mul(sv[:], teE[:], cs_bf[:])
                    nc.vector.tensor_add(hts_new[u][:], tdh[:], sv[:])
                    nc.vector.tensor_scalar_mul(
                        hT8b_new[:, BL:], hts_new[u][:], 1.0
                    )
                    # state update: deferred to the next step's start so the
                    # scheduler cannot slot it into the hT-critical tail.
                    pending_state.append((u, etqd, cs_bf, dh[0]))
                    return
                # s8 = e / den  (softmax gate, broadcast 1/den over m)
                s8 = wpp.tile([128, 7 * BL], bf16, name="wp", tag="wp")
                s8_v = s8.rearrange("p (m b) -> p m b", m=7)
                es_v = e_s[:, BL:].rearrange("p (m b) -> p m b", m=7)
                nc.vector.tensor_tensor(s8_v, es_v, cs_v, op=ALU.mult)
                if t > 0:
                    nc.vector.tensor_mul(s8[:], s8[:], tq[:])  # s*(q-h)
                    nc.vector.tensor_add(tq[:], hhat[u][:], s8[:])  # h + s(q-h)
                    nc.vector.tensor_mul(hhat[u][:], tq[:], decpat[:])
                else:
                    q_v = (
                        q_bf[:, u * BL : (u + 1) * BL]
                        .unsqueeze(1)
                        .broadcast_to([128, 7, BL])
                    )
                    nc.vector.tensor_tensor(s8_v, s8_v, q_v, op=ALU.mult)
                    nc.vector.tensor_mul(hhat[u][:], s8[:], decpat[:])
                tree7(hhat[u], hts_new[u])
                dst = hT8a_new if u < 2 else hT8b_new
                nc.vector.tensor_scalar_mul(
                    dst[:, (u % 2) * BL : (u % 2 + 1) * BL], hts_new[u][:], 1.0
                )

            e_tiles = {0: s_mm(0)}

            # ---------------- q phase ----------------
            q_bf = stp.tile([128, UT * BL], bf16, name="q_bf", bufs=1)
            for uq in range(UT):
                psq = pmm.tile([128, BL], f32, name="ps_mm", tag="ps_mm")
                if t > 0:
                    rhs_q = rhs_x + [
                        rh_bf[:, k * BL : (k + 1) * BL] for k in range(UT)
                    ]
                    mm_group(psq, wq_sb, uq * KT * 128, rhs_q, KT)
                else:
                    mm_group(psq, wq_sb, uq * KT * 128, rhs_x, 2)
                nc.scalar.activation(
                    q_bf[:, uq * BL : (uq + 1) * BL], psq[:],
                    AF.Tanh, bias=bq_sb[:, uq : uq + 1],
                )

            def make_tq(u):
                # DVE: q - h (broadcast q over the 7 m-slices)
                tq = tp.tile([128, 7 * BL], bf16, name="ehtq", tag="ehtq", bufs=2)
                tq_v = tq.rearrange("p (m b) -> p m b", m=7)
                q_v = (
                    q_bf[:, u * BL : (u + 1) * BL]
                    .unsqueeze(1)
                    .broadcast_to([128, 7, BL])
                )
                hh_v = hhat[u].rearrange("p (m b) -> p m b", m=7)
                nc.vector.tensor_tensor(tq_v, q_v, hh_v, op=ALU.subtract)
                return tq

            tqd3 = dh3 = etqd3 = te_tmp = None
            for u in range(UT):
                tq = make_tq(u) if (t > 0 and u < 3) else None
                e_s, dps = e_tiles.pop(u)
                s_el(u, e_s, dps, q_bf, tq,
                     etqd=etqd3 if (t > 0 and u == 3) else None, dh=dh3)
                if u + 1 < UT:
                    if t > 0 and u + 1 == 3:
                        # pre-decayed pieces of u3's state update (off the
                        # tail), emitted after s_el(2) so they can't delay it
                        tq3 = make_tq(3)
                        tqd3 = tp.tile([128, 7 * BL], bf16, name="tqd3", tag="tqd3", bufs=1)
                        nc.vector.tensor_mul(tqd3[:], tq3[:], decpat[:])
                        dh3t = tp.tile([128, 7 * BL], bf16, name="dh3", tag="dh3", bufs=1)
                        nc.vector.tensor_mul(dh3t[:], hhat[3][:], decpat[:])
                        etqd3 = tp.tile([128, 7 * BL], bf16, name="etqd3", tag="etqd3")
                        tdh3 = sp.tile([128, BL], bf16, name="tdh", tag="tdh", bufs=2)
                        tree7(dh3t, tdh3)
                        te1 = sp.tile([128, 3 * BL], bf16, name="te1", tag="te1", bufs=1)
                        te2 = sp.tile([128, 2 * BL], bf16, name="te2", tag="te2", bufs=1)
                        dh3 = (dh3t, (te1, te2, tdh3))
                        te_tmp = (te1, te2)
                    e_tiles[u + 1] = s_mm(
                        u + 1,
                        etqd=etqd3 if (t > 0 and u + 1 == 3) else None,
                        tqd=tqd3,
                    )

            hts = hts_new
            hT8 = (hT8a_new, hT8b_new)

        emit_y(T - 1, hts)

    nc.compile()
    _program_cache["nc"] = nc
    return nc


def _prep_shared(W_r, b_r, W_q, b_q, W_s, b_s, W_out):
    import ml_dtypes

    bf = ml_dtypes.bfloat16

    def perm_w(w):  # [768, 4096] -> [128, (m,u,k,c)]
        a = np.ascontiguousarray(w, np.float32).reshape(KT, 128, UT, 128, M)
        return np.ascontiguousarray(
            a.transpose(1, 4, 2, 0, 3).reshape(128, M * UT * KT * 128)
        ).astype(bf)

    from concourse import mybir

    f8np = mybir.dt.np(mybir.dt.float8e4)
    a = np.ascontiguousarray(W_r, np.float32).reshape(KT, 128, UT, 128, M)
    wr = np.ascontiguousarray(
        (a * 8.0).transpose(1, 4, 2, 0, 3).reshape(128, M * UT * KT * 128)
    ).astype(f8np)
    ws = perm_w(W_s)
    wq = np.ascontiguousarray(
        np.asarray(W_q, np.float32)
        .reshape(KT, 128, UT, 128)
        .transpose(1, 2, 0, 3)
        .reshape(128, UT * KT * 128)
    ).astype(bf)
    wo = np.ascontiguousarray(
        np.asarray(W_out, np.float32).reshape(UT, 128, 3).transpose(1, 0, 2).reshape(128, UT * 3)
    ).astype(bf)
    biasr = np.ascontiguousarray(
        (np.asarray(b_r, np.float32).reshape(UT, 128, M) - LN_TAU).transpose(1, 0, 2).reshape(128, UT * M)
    )
    biass = np.ascontiguousarray(
        (np.asarray(b_s, np.float32).reshape(UT, 128, M) - LN_TAU).transpose(1, 0, 2).reshape(128, UT * M)
    )
    biasq = np.ascontiguousarray(np.asarray(b_q, np.float32).reshape(UT, 128).T)
    ident = np.eye(128, dtype=np.float32).astype(bf)
    return dict(wr=wr, ws=ws, wq=wq, wo=wo, biasr=biasr, biass=biass, biasq=biasq,
                ident=ident, _f8np=f8np)


def kernel(x, W_r, b_r, W_q, b_q, W_s, b_s, W_out, b_out):
    _install_axon_hooks_shim()
    from concourse.bass_utils import run_bass_kernel_spmd

    import ml_dtypes

    bf = ml_dtypes.bfloat16

    nc = _build_program()
    shared = _prep_shared(W_r, b_r, W_q, b_q, W_s, b_s, W_out)

    x = np.asarray(x, np.float32)
    in_maps = []
    for c in range(N_CORES):
        xc = x[c * BL : (c + 1) * BL]  # [BL, T, F]
        xtf = np.ascontiguousarray(
            xc.transpose(1, 2, 0).reshape(T, 2, 128, BL).transpose(0, 2, 1, 3).reshape(T, 128, 2 * BL)
        )
        xt = xtf.astype(bf)
        xt8 = xtf.astype(shared["_f8np"])
        m = {k: v for k, v in shared.items() if k != "_f8np"}
        in_maps.append({"xt": xt, "xt8": xt8, **m})

    try:
        res = run_bass_kernel_spmd(nc, in_maps, list(range(N_CORES)))
    except Exception:
        # device pool may be wedged from an earlier crash — reset and retry
        try:
            lib = ctypes.CDLL("/opt/axon/libaxon_pjrt.so")
            lib.axon_reset.restype = ctypes.c_int64
            lib.axon_reset()
        except OSError:
            pass
        res = run_bass_kernel_spmd(nc, in_maps, list(range(N_CORES)))
    _program_cache["last_result"] = res

    out = np.empty((B, T, 3), np.float32)
    for c in range(N_CORES):
        y = res.results[c]["y"]  # [T, 3, BL]
        out[c * BL : (c + 1) * BL] = y.transpose(2, 0, 1)
    return out + np.asarray(b_out, np.float32)


# revision 30
# speedup vs baseline: 1.9136x; 1.0151x over previous
"""CTGRU forward kernel for 8 trn2 NeuronCores (data-parallel over batch).

Layout on device (per core, local batch BL=512):
  - All per-step tensors live as [U_partitions, m*BL + b] ("layout C", m-major
    free dim), so the hidden state h comes out of the m-reduction already in
    the [U, B] orientation the next step's matmuls need as their moving
    operand -> zero transposes inside the recurrence.
  - softmax(-(z - LN_TAU)^2) is computed as Derivative_Erf(z + (b - LN_TAU))
    = (2/sqrt(pi)) * exp(-d^2); the constant cancels in the normalization.
  - DECAY[0] == 0 exactly, so h_hat[..., m=0] is identically zero: the state
    stores only m=1..7 (7 slices) and all elementwise work skips m=0.
  - All DVE elementwise traffic is bf16 (2x mode); DECAY is folded into the
    per-m gate scale tile csd = DECAY_m/den so the state update collapses to
    one scalar_tensor_tensor per m: h' = h*DECAY_m + (e*csd)*(q-h).
"""

import contextlib
import ctypes
import sys
import types

import numpy as np

B, T, F, U, M = 4096, 16, 256, 512, 8
N_CORES = 8
BL = B // N_CORES  # 512
KT = (F + U) // 128  # 6 K-tiles over the fused dim
UT = U // 128  # 4 u-tiles

LN_TAU = (np.arange(M, dtype=np.float32) * (0.5 * np.log(10.0))).astype(np.float32)
DECAY = np.exp(-0.04 / (LN_TAU + 1e-7)).astype(np.float32)  # DECAY[0] == 0.0


def _install_axon_hooks_shim():
    """Make `antenv.axon_hooks` importable when the image lacks it, so
    BASS_TRACE-triggered profiling in run_bass_kernel_spmd can't crash us."""
    name = "antenv.axon_hooks"
    if name in sys.modules:
        return
    so_path = "/opt/axon/libaxon_pjrt.so"

    def _build_hook():
        try:
            lib = ctypes.CDLL(so_path)
        except OSError:
            return None
        if not hasattr(lib, "axon_start_nrt_profile"):
            return None
        lib.axon_start_nrt_profile.argtypes = [
            ctypes.POINTER(ctypes.c_int64),
            ctypes.c_size_t,
        ]
        lib.axon_start_nrt_profile.restype = ctypes.c_int64
        lib.axon_stop_nrt_profile.argtypes = [ctypes.c_char_p]
        lib.axon_stop_nrt_profile.restype = ctypes.c_int64

        @contextlib.contextmanager
        def _hook(output_dir, device_ids):
            import jax

            jax.devices()
            if device_ids:
                ids = (ctypes.c_int64 * len(device_ids))(*device_ids)
                rc = lib.axon_start_nrt_profile(ids, len(device_ids))
            else:
                rc = lib.axon_start_nrt_profile(None, 0)
            if rc != 0:
                raise RuntimeError(f"axon_start_nrt_profile rc={rc}")
            try:
                yield
            finally:
                n = lib.axon_stop_nrt_profile(str(output_dir).encode())
                print(f"profile: {n} file(s) written to {output_dir}", file=sys.stderr)

        return _hook

    mod = types.ModuleType(name)
    holder = [_build_hook()]
    mod.get_axon_ntff_profile_hook = lambda: holder[0]
    mod.set_axon_ntff_profile_hook = lambda h: holder.__setitem__(0, h)
    sys.modules[name] = mod
    try:
        import antenv

        antenv.axon_hooks = mod
    except ImportError:
        pass


_program_cache = {}


def _build_program():
    if "nc" in _program_cache:
        return _program_cache["nc"]

    import concourse.bass as bass
    import concourse.tile as tile
    from concourse import bacc, mybir

    f32 = mybir.dt.float32
    bf16 = mybir.dt.bfloat16
    AF = mybir.ActivationFunctionType
    ALU = mybir.AluOpType

    nc = bacc.Bacc("TRN2", target_bir_lowering=False, debug=False)

    f8 = mybir.dt.float8e4
    xt_d = nc.dram_tensor("xt", [T, 128, 2 * BL], bf16, kind="ExternalInput").ap()
    xt8_d = nc.dram_tensor("xt8", [T, 128, 2 * BL], f8, kind="ExternalInput").ap()
    id_d = nc.dram_tensor("ident", [128, 128], bf16, kind="ExternalInput").ap()
    wr_d = nc.dram_tensor("wr", [128, M * UT * KT * 128], f8, kind="ExternalInput").ap()
    ws_d = nc.dram_tensor("ws", [128, M * UT * KT * 128], bf16, kind="ExternalInput").ap()
    wq_d = nc.dram_tensor("wq", [128, UT * KT * 128], bf16, kind="ExternalInput").ap()
    wo_d = nc.dram_tensor("wo", [128, UT * 3], bf16, kind="ExternalInput").ap()
    br_d = nc.dram_tensor("biasr", [128, UT * M], f32, kind="ExternalInput").ap()
    bs_d = nc.dram_tensor("biass", [128, UT * M], f32, kind="ExternalInput").ap()
    bq_d = nc.dram_tensor("biasq", [128, UT], f32, kind="ExternalInput").ap()
    y_d = nc.dram_tensor("y", [T, 3, BL], f32, kind="ExternalOutput").ap()

    with tile.TileContext(nc) as tc, contextlib.ExitStack() as ctx:
        const = ctx.enter_context(tc.tile_pool(name="const", bufs=1))
        state = ctx.enter_context(tc.tile_pool(name="state", bufs=1))
        wsp = ctx.enter_context(tc.tile_pool(name="wsp", bufs=6))
        rtp = ctx.enter_context(tc.tile_pool(name="rtp", bufs=1))
        xp = ctx.enter_context(tc.tile_pool(name="xp", bufs=3))
        ep = ctx.enter_context(tc.tile_pool(name="ep", bufs=2))
        tp = ctx.enter_context(tc.tile_pool(name="tp", bufs=2))
        wpp = ctx.enter_context(tc.tile_pool(name="wpp", bufs=1))
        sp = ctx.enter_context(tc.tile_pool(name="sp", bufs=2))
        stp = ctx.enter_context(tc.tile_pool(name="stp", bufs=2))
        htp = ctx.enter_context(tc.tile_pool(name="htp", bufs=2))
        pmm = ctx.enter_context(tc.tile_pool(name="pmm", bufs=5, space="PSUM"))
        py = ctx.enter_context(tc.tile_pool(name="py", bufs=1, space="PSUM"))
        pden = ctx.enter_context(tc.tile_pool(name="pden", bufs=2, space="PSUM"))

        # ---- weight / bias preload ----
        wr_sb = const.tile([128, M * UT * KT * 128], f8, name="wr_sb")
        for m in range(M):
            sl = slice(m * UT * KT * 128, (m + 1) * UT * KT * 128)
            nc.sync.dma_start(wr_sb[:, sl], wr_d[:, sl])
        wq_sb = const.tile([128, UT * KT * 128], bf16, name="wq_sb")
        nc.sync.dma_start(wq_sb[:], wq_d[:])
        id_sb = const.tile([128, 128], bf16, name="id_sb")
        nc.sync.dma_start(id_sb[:], id_d[:])
        wo_sb = const.tile([128, UT * 3], bf16, name="wo_sb")
        nc.sync.dma_start(wo_sb[:], wo_d[:])
        br_sb = const.tile([128, UT * M], f32, name="br_sb")
        nc.sync.dma_start(br_sb[:], br_d[:])
        bs_sb = const.tile([128, UT * M], f32, name="bs_sb")
        nc.sync.dma_start(bs_sb[:], bs_d[:])
        bq_sb = const.tile([128, UT], f32, name="bq_sb")
        nc.sync.dma_start(bq_sb[:], bq_d[:])

        # DECAY pattern tile: slice mi holds DECAY[mi+1] (m=1..7)
        decpat = const.tile([128, 7 * BL], bf16, name="decpat")
        for mi in range(7):
            nc.vector.memset(decpat[:, mi * BL : (mi + 1) * BL], float(DECAY[mi + 1]))

        # h_hat state: m-slices 1..7 only (slice 0 is identically zero)
        hhat = [
            state.tile([128, 7 * BL], bf16, name=f"hhat{u}", tag=f"hhat{u}")
            for u in range(UT)
        ]

        hts = None  # list of 4 [128, BL] bf16 tiles: h(t-1) per u-tile
        hT8 = None  # [128, UT*BL] fp8 copy of h(t-1) for the r-phase matmuls

        def tree8(src, out):
            """out[128,BL] = sum of 8 contiguous BL-slices of src (bf16 tree)."""
            t1 = rtp.tile([128, 4 * BL], bf16, name="rt1", tag="rt1")
            t2 = rtp.tile([128, 2 * BL], bf16, name="rt2", tag="rt2")
            nc.vector.tensor_add(t1[:], src[:, : 4 * BL], src[:, 4 * BL :])
            nc.vector.tensor_add(t2[:], t1[:, : 2 * BL], t1[:, 2 * BL :])
            nc.vector.tensor_add(out[:], t2[:, :BL], t2[:, BL:])

        def tree7(src, out):
            """out[128,BL] = sum of 7 contiguous BL-slices of src (bf16 tree)."""
            t1 = rtp.tile([128, 4 * BL], bf16, name="rt1", tag="rt1")
            t2 = rtp.tile([128, 2 * BL], bf16, name="rt2", tag="rt2")
            nc.vector.tensor_add(t1[:, : 3 * BL], src[:, : 3 * BL], src[:, 4 * BL : 7 * BL])
            nc.vector.tensor_add(t2[:, :BL], t1[:, :BL], t1[:, BL : 2 * BL])
            nc.vector.tensor_add(
                t2[:, BL : 2 * BL], t1[:, 2 * BL : 3 * BL], src[:, 3 * BL : 4 * BL]
            )
            nc.vector.tensor_add(out[:], t2[:, :BL], t2[:, BL : 2 * BL])

        def mm_group_r8(ps, base, xt8_t, hT8, head=True, tail=True):
            """r-phase fused matmul in fp8 DoubleRow: 3 instrs cover K=768."""
            hT8a, hT8b = hT8
            pairs = [
                xt8_t.rearrange("p (two b) -> p two b", two=2),
                hT8a.rearrange("p (two b) -> p two b", two=2),
                hT8b.rearrange("p (two b) -> p two b", two=2),
            ]
            js = ([0, 1] if head else []) + ([2] if tail else [])
            for j in js:
                lhsT = wr_sb[:, base + j * 256 : base + (j + 1) * 256].rearrange(
                    "p (two c) -> p two c", two=2
                )
                nc.tensor.matmul(
                    ps[:], lhsT, pairs[j],
                    start=(j == 0), stop=(j == 2),
                    perf_mode=mybir.MatmulPerfMode.DoubleRow,
                )

        def den_mm(dps, e_tile, m):
            """Accumulate e m-slice into the den psum via an identity matmul."""
            nc.tensor.matmul(
                dps[:], id_sb[:], e_tile[:, m * BL : (m + 1) * BL],
                start=(m == 0), stop=(m == M - 1),
            )

        def mm_group(ps, w_sb, base, rhs, n_k):
            """Accumulate the fused matmul into psum `ps`. rhs: list of K-tile
            moving operands (x k-tiles, then h k-tiles)."""
            for k in range(n_k):
                nc.tensor.matmul(
                    ps[:],
                    w_sb[:, base + k * 128 : base + (k + 1) * 128],
                    rhs[k],
                    start=(k == 0), stop=(k == n_k - 1),
                )

        def emit_y(t, h_tiles):
            psy = py.tile([3, BL], f32, name="ps_y", tag="ps_y")
            for k in range(UT):
                nc.tensor.matmul(
                    psy[:], wo_sb[:, k * 3 : (k + 1) * 3], h_tiles[k][:],
                    start=(k == 0), stop=(k == UT - 1),
                )
            y_sb = stp.tile([3, BL], f32, name="y_sb")
            nc.scalar.copy(y_sb[:], psy[:])
            nc.sync.dma_start(y_d[t], y_sb[:])

        pending_state = []

        def flush_state():
            while pending_state:
                u, etqd, cs_bf, dh0 = pending_state.pop()
                # per-m quanta: small enough that scheduler insertions into
                # the boundary-critical tail cost <0.4us each, not 1.9us
                with tc.high_priority(offset=-150):
                    for mi in range(7):
                        sl = slice(mi * BL, (mi + 1) * BL)
                        nc.vector.tensor_mul(etqd[:, sl], etqd[:, sl], cs_bf[:])
                        nc.vector.tensor_add(hhat[u][:, sl], dh0[:, sl], etqd[:, sl])

        for t in range(T):
            xt_t = xp.tile([128, 2 * BL], bf16, name="xt_t", tag="xt")
            nc.sync.dma_start(xt_t[:], xt_d[t])
            xt8_t = xp.tile([128, 2 * BL], f8, name="xt8_t", tag="xt8")
            nc.sync.dma_start(xt8_t[:], xt8_d[t])
            flush_state()
            rhs_x = [xt_t[:, 0:BL], xt_t[:, BL : 2 * BL]]
            rhs_full = rhs_x + [h[:] for h in hts] if t > 0 else None

                rh_bf = stp.tile([128, UT * BL], bf16, name="rh_bf", bufs=1)
                for u in range(UT):
                    e_r = ep.tile([128, M * BL], bf16, name="e_t", tag="e_t")
                    dps = pden.tile([128, BL], f32, name="ps_den", tag="ps_den")
                    pss = {}
                    if u == 0:
                        # run-ahead: x/h01 pairs of the first 4 groups fill
                        # the PE while u3's elementwise tail finishes h.
                        for m in range(4):
                            pss[m] = pmm.tile([128, BL], f32, name="ps_mm", tag="ps_mm")
                            base = (m * UT + u) * KT * 128
                            mm_group_r8(pss[m], base, xt8_t, hT8, tail=False)
                    for m in range(M):
                        base = (m * UT + u) * KT * 128
                        if m in pss:
                            ps = pss[m]
                            mm_group_r8(ps, base, xt8_t, hT8, head=False)
                        else:
                            ps = pmm.tile([128, BL], f32, name="ps_mm", tag="ps_mm")
                            mm_group_r8(ps, base, xt8_t, hT8)
                        nc.scalar.activation(
                            e_r[:, m * BL : (m + 1) * BL], ps[:],
                            AF.Derivative_Erf, scale=0.125,
                            bias=br_sb[:, u * M + m : u * M + m + 1],
                        )
                        if m >= 2:
                            den_mm(dps, e_r, m - 2)
                    if u == 0:
                        # PE: y-phase of the previous step (hT ready, fills
                        # the pipeline right after the first r-group).
                        emit_y(t - 1, hts)
                    den_mm(dps, e_r, M - 2)
                    den_mm(dps, e_r, M - 1)
                    # DVE: e*h products + weighted-sum tree + normalize
                    eh = tp.tile([128, 7 * BL], bf16, name="ehtq", tag="ehtq", bufs=2)
                    nc.vector.tensor_mul(eh[:], e_r[:, BL:], hhat[u][:])
                    rhn = sp.tile([128, BL], bf16, name="rhn", tag="rhn", bufs=1)
                    tree7(eh, rhn)
                    cr = sp.tile([128, BL], f32, name="crec", tag="crec", bufs=2)
                    nc.vector.reciprocal_approx_fast(out=cr[:], in_=dps[:])
                    nc.vector.tensor_mul(
                        rh_bf[:, u * BL : (u + 1) * BL], rhn[:], cr[:]
                    )            # ---------------- r phase (t=0: h_hat==0 makes r irrelevant) ----
            if t > 0:
                rh_bf = stp.tile([128, UT * BL], bf16, name="rh_bf", bufs=1)
                for u in range(UT):
                    e_r = ep.tile([128, M * BL], bf16, name="e_t", tag="e_t")
                    dps = pden.tile([128, BL], f32, name="ps_den", tag="ps_den")
                    pss = {}
                    if u == 0:
                        # run-ahead: x/h01 pairs of the first 4 groups fill
                        # the PE while u3's elementwise tail finishes h.
                        for m in range(4):
                            pss[m] = pmm.tile([128, BL], f32, name="ps_mm", tag="ps_mm")
                            base = (m * UT + u) * KT * 128
                            mm_group_r8(pss[m], base, xt8_t, hT8, tail=False)
                    for m in range(M):
                        base = (m * UT + u) * KT * 128
                        if m in pss:
                            ps = pss[m]
                            mm_group_r8(ps, base, xt8_t, hT8, head=False)
                        else:
                            ps = pmm.tile([128, BL], f32, name="ps_mm", tag="ps_mm")
                            mm_group_r8(ps, base, xt8_t, hT8)
                        nc.scalar.activation(
                            e_r[:, m * BL : (m + 1) * BL], ps[:],
                            AF.Derivative_Erf, scale=0.125,
                            bias=br_sb[:, u * M + m : u * M + m + 1],
                        )
                        if m >= 2:
                            den_mm(dps, e_r, m - 2)
                    if u == 0:
                        # PE: y-phase of the previous step (hT ready, fills
                        # the pipeline right after the first r-group).
                        emit_y(t - 1, hts)
                    den_mm(dps, e_r, M - 2)
                    den_mm(dps, e_r, M - 1)
                    # DVE: e*h products + weighted-sum tree + normalize
                    eh = tp.tile([128, 7 * BL], bf16, name="ehtq", tag="ehtq", bufs=2)
                    nc.vector.tensor_mul(eh[:], e_r[:, BL:], hhat[u][:])
                    rhn = sp.tile([128, BL], bf16, name="rhn", tag="rhn", bufs=1)
                    tree7(eh, rhn)
                    cr = sp.tile([128, BL], f32, name="crec", tag="crec", bufs=2)
                    nc.vector.reciprocal_approx_fast(out=cr[:], in_=dps[:])
                    nc.vector.tensor_mul(
                        rh_bf[:, u * BL : (u + 1) * BL], rhn[:], cr[:]
                    )

            # ------- s phase matmuls / elementwise, interleaved with q -----
            hts_new = [
                htp.tile([128, BL], bf16, name=f"hT{u}", tag=f"hT{u}")
                for u in range(UT)
            ]
            hT8a_new = htp.tile([128, 2 * BL], f8, name="hT8a", tag="hT8a")
            hT8b_new = htp.tile([128, 2 * BL], f8, name="hT8b", tag="hT8b")

            def s_mm(u, etqd=None, tqd=None, te=None):
                e_s = ep.tile([128, M * BL], bf16, name="e_t", tag="e_t")
                dps = pden.tile([128, BL], f32, name="ps_den", tag="ps_den")
                for m in range(M):
                    wsch = wsp.tile([128, KT * 128], bf16, name="wsch")
                    base = (m * UT + u) * KT * 128
                    nc.sync.dma_start(wsch[:], ws_d[:, base : base + KT * 128])
                    ps = pmm.tile([128, BL], f32, name="ps_mm", tag="ps_mm")
                    rhs = rhs_full if t > 0 else rhs_x
                    mm_group(ps, wsch, 0, rhs, KT if t > 0 else 2)
                    nc.scalar.activation(
                        e_s[:, m * BL : (m + 1) * BL], ps[:],
                        AF.Derivative_Erf,
                        bias=bs_sb[:, u * M + m : u * M + m + 1],
                    )
                    if m >= 2:
                        den_mm(dps, e_s, m - 2)
                    if etqd is not None and m >= 1:
                        sl = slice((m - 1) * BL, m * BL)
                        nc.vector.tensor_mul(
                            etqd[:, sl], e_s[:, m * BL : (m + 1) * BL], tqd[:, sl]
                        )
                        te1, te2 = te
                        if m == 3:
                            nc.vector.tensor_add(
                                te1[:, :BL], etqd[:, :BL], etqd[:, BL : 2 * BL]
                            )
                        elif m == 5:
                            nc.vector.tensor_add(
                                te1[:, BL : 2 * BL],
                                etqd[:, 2 * BL : 3 * BL], etqd[:, 3 * BL : 4 * BL],
                            )
                        elif m == 7:
                            nc.vector.tensor_add(
                                te1[:, 2 * BL : 3 * BL],
                                etqd[:, 4 * BL : 5 * BL], etqd[:, 5 * BL : 6 * BL],
                            )
                            nc.vector.tensor_add(
                                te2[:, :BL], te1[:, :BL], te1[:, BL : 2 * BL]
                            )
                den_mm(dps, e_s, M - 2)
                den_mm(dps, e_s, M - 1)
                return e_s, dps

            def s_el(u, e_s, dps, q_bf, tq, etqd=None, dh=None):
                cs = sp.tile([128, BL], f32, name="crec", tag="crec", bufs=2)
                nc.vector.reciprocal_approx_fast(out=cs[:], in_=dps[:])
                cs_bf = sp.tile([128, BL], bf16, name="csbf", tag="csbf", bufs=2)
                nc.vector.tensor_scalar_mul(cs_bf[:], cs[:], 1.0)
                cs_v = cs_bf.unsqueeze(1).broadcast_to([128, 7, BL])
                if etqd is not None:
                    # fast tail (last u): hT = tree(h*D) + tree(e*tq*D)/den;
                    # only 512-wide ops remain after the last activation.
                    te1, te2, tdh = dh[1]
                    nc.vector.tensor_add(
                        te2[:, BL : 2 * BL], te1[:, 2 * BL : 3 * BL], etqd[:, 6 * BL :]
                    )
                    teE = sp.tile([128, BL], bf16, name="teE", tag="teE", bufs=1)
                    nc.vector.tensor_add(teE[:], te2[:, :BL], te2[:, BL : 2 * BL])
                    sv = sp.tile([128, BL], bf16, name="sv", tag="sv", bufs=1)
                    nc.vector.tensor_mul(sv[:], teE[:], cs_bf[:])
                    nc.vector.tensor_add(hts_new[u][:], tdh[:], sv[:])
                    nc.vector.tensor_scalar_mul(
                        hT8b_new[:, (u % 2) * BL : (u % 2 + 1) * BL],
                        hts_new[u][:], 1.0,
                    )
                    # state update: deferred to the next step's start so the
                    # scheduler cannot slot it into the hT-critical tail.
                    pending_state.append((u, etqd, cs_bf, dh[0]))
                    return
                # s8 = e / den  (softmax gate, broadcast 1/den over m)
                s8 = wpp.tile([128, 7 * BL], bf16, name="wp", tag="wp")
                s8_v = s8.rearrange("p (m b) -> p m b", m=7)
                es_v = e_s[:, BL:].rearrange("p (m b) -> p m b", m=7)
                nc.vector.tensor_tensor(s8_v, es_v, cs_v, op=ALU.mult)
                if t > 0:
                    nc.vector.tensor_mul(s8[:], s8[:], tq[:])  # s*(q-h)
                    nc.vector.tensor_add(tq[:], hhat[u][:], s8[:])  # h + s(q-h)
                    nc.vector.tensor_mul(hhat[u][:], tq[:], decpat[:])
                else:
                    q_v = (
                        q_bf[:, u * BL : (u + 1) * BL]
                        .unsqueeze(1)
                        .broadcast_to([128, 7, BL])
                    )
                    nc.vector.tensor_tensor(s8_v, s8_v, q_v, op=ALU.mult)
                    nc.vector.tensor_mul(hhat[u][:], s8[:], decpat[:])
                tree7(hhat[u], hts_new[u])
                dst = hT8a_new if u < 2 else hT8b_new
                nc.vector.tensor_scalar_mul(
                    dst[:, (u % 2) * BL : (u % 2 + 1) * BL], hts_new[u][:], 1.0
                )

            e_tiles = {0: s_mm(0)}

            # ---------------- q phase ----------------
            q_bf = stp.tile([128, UT * BL], bf16, name="q_bf", bufs=1)
            for uq in range(UT):
                psq = pmm.tile([128, BL], f32, name="ps_mm", tag="ps_mm")
                if t > 0:
                    rhs_q = rhs_x + [
                        rh_bf[:, k * BL : (k + 1) * BL] for k in range(UT)
                    ]
                    mm_group(psq, wq_sb, uq * KT * 128, rhs_q, KT)
                else:
                    mm_group(psq, wq_sb, uq * KT * 128, rhs_x, 2)
                nc.scalar.activation(
                    q_bf[:, uq * BL : (uq + 1) * BL], psq[:],
                    AF.Tanh, bias=bq_sb[:, uq : uq + 1],
                )

            def make_tq(u):
                # DVE: q - h (broadcast q over the 7 m-slices)
                tq = tp.tile([128, 7 * BL], bf16, name="ehtq", tag="ehtq", bufs=2)
                tq_v = tq.rearrange("p (m b) -> p m b", m=7)
                q_v = (
                    q_bf[:, u * BL : (u + 1) * BL]
                    .unsqueeze(1)
                    .broadcast_to([128, 7, BL])
                )
                hh_v = hhat[u].rearrange("p (m b) -> p m b", m=7)
                nc.vector.tensor_tensor(tq_v, q_v, hh_v, op=ALU.subtract)
                return tq

            fast = {}
            for u in range(UT):
                tq = make_tq(u) if (t > 0 and u < 2) else None
                e_s, dps = e_tiles.pop(u)
                fu = fast.get(u)
                s_el(u, e_s, dps, q_bf, tq,
                     etqd=fu[0] if fu else None, dh=fu[1] if fu else None)
                if u + 1 < UT:
                    if t > 0 and u + 1 >= 2:
                        # pre-decayed pieces of the last two u-tiles' state
                        # updates (off the tail), emitted after s_el(u) so
                        # they cannot delay it
                        un = u + 1
                        tqn = make_tq(un)
                        tqd = tp.tile([128, 7 * BL], bf16, name="tqd3", tag="tqd3", bufs=1)
                        nc.vector.tensor_mul(tqd[:], tqn[:], decpat[:])
                        dht = tp.tile([128, 7 * BL], bf16, name="dh3", tag="dh3", bufs=2)
                        nc.vector.tensor_mul(dht[:], hhat[un][:], decpat[:])
                        etqd = tp.tile([128, 7 * BL], bf16, name="etqd3", tag="etqd3")
                        tdh = sp.tile([128, BL], bf16, name="tdh", tag="tdh", bufs=2)
                        tree7(dht, tdh)
                        te1 = sp.tile([128, 3 * BL], bf16, name="te1", tag="te1", bufs=1)
                        te2 = sp.tile([128, 2 * BL], bf16, name="te2", tag="te2", bufs=1)
                        fast[un] = (etqd, (dht, (te1, te2, tdh)))
                        e_tiles[un] = s_mm(un, etqd=etqd, tqd=tqd, te=(te1, te2))
                    else:
                        e_tiles[u + 1] = s_mm(u + 1)

            hts = hts_new
            hT8 = (hT8a_new, hT8b_new)

        emit_y(T - 1, hts)

    nc.compile()
    _program_cache["nc"] = nc
    return nc


def _prep_shared(W_r, b_r, W_q, b_q, W_s, b_s, W_out):
    import ml_dtypes

    bf = ml_dtypes.bfloat16

    def perm_w(w):  # [768, 4096] -> [128, (m,u,k,c)]
        a = np.ascontiguousarray(w, np.float32).reshape(KT, 128, UT, 128, M)
        return np.ascontiguousarray(
            a.transpose(1, 4, 2, 0, 3).reshape(128, M * UT * KT * 128)
        ).astype(bf)

    from concourse import mybir

    f8np = mybir.dt.np(mybir.dt.float8e4)
    a = np.ascontiguousarray(W_r, np.float32).reshape(KT, 128, UT, 128, M)
    wr = np.ascontiguousarray(
        (a * 8.0).transpose(1, 4, 2, 0, 3).reshape(128, M * UT * KT * 128)
    ).astype(f8np)
    ws = perm_w(W_s)
    wq = np.ascontiguousarray(
        np.asarray(W_q, np.float32)
        .reshape(KT, 128, UT, 128)
        .transpose(1, 2, 0, 3)
        .reshape(128, UT * KT * 128)
    ).astype(bf)
    wo = np.ascontiguousarray(
        np.asarray(W_out, np.float32).reshape(UT, 128, 3).transpose(1, 0, 2).reshape(128, UT * 3)
    ).astype(bf)
    biasr = np.ascontiguousarray(
        (np.asarray(b_r, np.float32).reshape(UT, 128, M) - LN_TAU).transpose(1, 0, 2).reshape(128, UT * M)
    )
    biass = np.ascontiguousarray(
        (np.asarray(b_s, np.float32).reshape(UT, 128, M) - LN_TAU).transpose(1, 0, 2).reshape(128, UT * M)
    )
    biasq = np.ascontiguousarray(np.asarray(b_q, np.float32).reshape(UT, 128).T)
    ident = np.eye(128, dtype=np.float32).astype(bf)
    return dict(wr=wr, ws=ws, wq=wq, wo=wo, biasr=biasr, biass=biass, biasq=biasq,
                ident=ident, _f8np=f8np)


def kernel(x, W_r, b_r, W_q, b_q, W_s, b_s, W_out, b_out):
    _install_axon_hooks_shim()
    from concourse.bass_utils import run_bass_kernel_spmd

    import ml_dtypes

    bf = ml_dtypes.bfloat16

    nc = _build_program()
    shared = _prep_shared(W_r, b_r, W_q, b_q, W_s, b_s, W_out)

    x = np.asarray(x, np.float32)
    in_maps = []
    for c in range(N_CORES):
        xc = x[c * BL : (c + 1) * BL]  # [BL, T, F]
        xtf = np.ascontiguousarray(
            xc.transpose(1, 2, 0).reshape(T, 2, 128, BL).transpose(0, 2, 1, 3).reshape(T, 128, 2 * BL)
        )
        xt = xtf.astype(bf)
        xt8 = xtf.astype(shared["_f8np"])
        m = {k: v for k, v in shared.items() if k != "_f8np"}
        in_maps.append({"xt": xt, "xt8": xt8, **m})

    try:
        res = run_bass_kernel_spmd(nc, in_maps, list(range(N_CORES)))
    except Exception:
        # device pool may be wedged from an earlier crash — reset and retry
        try:
            lib = ctypes.CDLL("/opt/axon/libaxon_pjrt.so")
            lib.axon_reset.restype = ctypes.c_int64
            lib.axon_reset()
        except OSError:
            pass
        res = run_bass_kernel_spmd(nc, in_maps, list(range(N_CORES)))
    _program_cache["last_result"] = res

    out = np.empty((B, T, 3), np.float32)
    for c in range(N_CORES):
        y = res.results[c]["y"]  # [T, 3, BL]
        out[c * BL : (c + 1) * BL] = y.transpose(2, 0, 1)
    return out + np.asarray(b_out, np.float32)


# revision 31
# speedup vs baseline: 1.9542x; 1.0212x over previous
"""CTGRU forward kernel for 8 trn2 NeuronCores (data-parallel over batch).

Layout on device (per core, local batch BL=512):
  - All per-step tensors live as [U_partitions, m*BL + b] ("layout C", m-major
    free dim), so the hidden state h comes out of the m-reduction already in
    the [U, B] orientation the next step's matmuls need as their moving
    operand -> zero transposes inside the recurrence.
  - softmax(-(z - LN_TAU)^2) is computed as Derivative_Erf(z + (b - LN_TAU))
    = (2/sqrt(pi)) * exp(-d^2); the constant cancels in the normalization.
  - DECAY[0] == 0 exactly, so h_hat[..., m=0] is identically zero: the state
    stores only m=1..7 (7 slices) and all elementwise work skips m=0.
  - All DVE elementwise traffic is bf16 (2x mode); DECAY is folded into the
    per-m gate scale tile csd = DECAY_m/den so the state update collapses to
    one scalar_tensor_tensor per m: h' = h*DECAY_m + (e*csd)*(q-h).
"""

import contextlib
import ctypes
import sys
import types

import numpy as np

B, T, F, U, M = 4096, 16, 256, 512, 8
N_CORES = 8
BL = B // N_CORES  # 512
KT = (F + U) // 128  # 6 K-tiles over the fused dim
UT = U // 128  # 4 u-tiles

LN_TAU = (np.arange(M, dtype=np.float32) * (0.5 * np.log(10.0))).astype(np.float32)
DECAY = np.exp(-0.04 / (LN_TAU + 1e-7)).astype(np.float32)  # DECAY[0] == 0.0


def _install_axon_hooks_shim():
    """Make `antenv.axon_hooks` importable when the image lacks it, so
    BASS_TRACE-triggered profiling in run_bass_kernel_spmd can't crash us."""
    name = "antenv.axon_hooks"
    if name in sys.modules:
        return
    so_path = "/opt/axon/libaxon_pjrt.so"

    def _build_hook():
        try:
            lib = ctypes.CDLL(so_path)
        except OSError:
            return None
        if not hasattr(lib, "axon_start_nrt_profile"):
            return None
        lib.axon_start_nrt_profile.argtypes = [
            ctypes.POINTER(ctypes.c_int64),
            ctypes.c_size_t,
        ]
        lib.axon_start_nrt_profile.restype = ctypes.c_int64
        lib.axon_stop_nrt_profile.argtypes = [ctypes.c_char_p]
        lib.axon_stop_nrt_profile.restype = ctypes.c_int64

        @contextlib.contextmanager
        def _hook(output_dir, device_ids):
            import jax

            jax.devices()
            if device_ids:
                ids = (ctypes.c_int64 * len(device_ids))(*device_ids)
                rc = lib.axon_start_nrt_profile(ids, len(device_ids))
            else:
                rc = lib.axon_start_nrt_profile(None, 0)
            if rc != 0:
                raise RuntimeError(f"axon_start_nrt_profile rc={rc}")
            try:
                yield
            finally:
                n = lib.axon_stop_nrt_profile(str(output_dir).encode())
                print(f"profile: {n} file(s) written to {output_dir}", file=sys.stderr)

        return _hook

    mod = types.ModuleType(name)
    holder = [_build_hook()]
    mod.get_axon_ntff_profile_hook = lambda: holder[0]
    mod.set_axon_ntff_profile_hook = lambda h: holder.__setitem__(0, h)
    sys.modules[name] = mod
    try:
        import antenv

        antenv.axon_hooks = mod
    except ImportError:
        pass


_program_cache = {}


def _build_program():
    if "nc" in _program_cache:
        return _program_cache["nc"]

    import concourse.bass as bass
    import concourse.tile as tile
    from concourse import bacc, mybir

    f32 = mybir.dt.float32
    bf16 = mybir.dt.bfloat16
    AF = mybir.ActivationFunctionType
    ALU = mybir.AluOpType

    nc = bacc.Bacc("TRN2", target_bir_lowering=False, debug=False)

    f8 = mybir.dt.float8e4
    xt_d = nc.dram_tensor("xt", [T, 128, 2 * BL], bf16, kind="ExternalInput").ap()
    xt8_d = nc.dram_tensor("xt8", [T, 128, 2 * BL], f8, kind="ExternalInput").ap()
    id_d = nc.dram_tensor("ident", [128, 128], bf16, kind="ExternalInput").ap()
    wr_d = nc.dram_tensor("wr", [128, M * UT * KT * 128], f8, kind="ExternalInput").ap()
    ws_d = nc.dram_tensor("ws", [128, M * UT * KT * 128], bf16, kind="ExternalInput").ap()
    wq_d = nc.dram_tensor("wq", [128, UT * KT * 128], bf16, kind="ExternalInput").ap()
    wo_d = nc.dram_tensor("wo", [128, UT * 3], bf16, kind="ExternalInput").ap()
    br_d = nc.dram_tensor("biasr", [128, UT * M], f32, kind="ExternalInput").ap()
    bs_d = nc.dram_tensor("biass", [128, UT * M], f32, kind="ExternalInput").ap()
    bq_d = nc.dram_tensor("biasq", [128, UT], f32, kind="ExternalInput").ap()
    y_d = nc.dram_tensor("y", [T, 3, BL], f32, kind="ExternalOutput").ap()

    with tile.TileContext(nc) as tc, contextlib.ExitStack() as ctx:
        const = ctx.enter_context(tc.tile_pool(name="const", bufs=1))
        state = ctx.enter_context(tc.tile_pool(name="state", bufs=1))
        wsp = ctx.enter_context(tc.tile_pool(name="wsp", bufs=4))
        rtp = ctx.enter_context(tc.tile_pool(name="rtp", bufs=1))
        xp = ctx.enter_context(tc.tile_pool(name="xp", bufs=3))
        ep = ctx.enter_context(tc.tile_pool(name="ep", bufs=2))
        tp = ctx.enter_context(tc.tile_pool(name="tp", bufs=2))
        wpp = ctx.enter_context(tc.tile_pool(name="wpp", bufs=1))
        sp = ctx.enter_context(tc.tile_pool(name="sp", bufs=2))
        stp = ctx.enter_context(tc.tile_pool(name="stp", bufs=2))
        htp = ctx.enter_context(tc.tile_pool(name="htp", bufs=2))
        pmm = ctx.enter_context(tc.tile_pool(name="pmm", bufs=5, space="PSUM"))
        py = ctx.enter_context(tc.tile_pool(name="py", bufs=1, space="PSUM"))
        pden = ctx.enter_context(tc.tile_pool(name="pden", bufs=2, space="PSUM"))

        # ---- weight / bias preload ----
        wr_sb = const.tile([128, M * UT * KT * 128], f8, name="wr_sb")
        for m in range(M):
            sl = slice(m * UT * KT * 128, (m + 1) * UT * KT * 128)
            nc.sync.dma_start(wr_sb[:, sl], wr_d[:, sl])
        wq_sb = const.tile([128, UT * KT * 128], bf16, name="wq_sb")
        nc.sync.dma_start(wq_sb[:], wq_d[:])
        id_sb = const.tile([128, 128], bf16, name="id_sb")
        nc.sync.dma_start(id_sb[:], id_d[:])
        wo_sb = const.tile([128, UT * 3], bf16, name="wo_sb")
        nc.sync.dma_start(wo_sb[:], wo_d[:])
        br_sb = const.tile([128, UT * M], f32, name="br_sb")
        nc.sync.dma_start(br_sb[:], br_d[:])
        bs_sb = const.tile([128, UT * M], f32, name="bs_sb")
        nc.sync.dma_start(bs_sb[:], bs_d[:])
        bq_sb = const.tile([128, UT], f32, name="bq_sb")
        nc.sync.dma_start(bq_sb[:], bq_d[:])

        # DECAY pattern tile: slice mi holds DECAY[mi+1] (m=1..7)
        decpat = const.tile([128, 7 * BL], bf16, name="decpat")
        for mi in range(7):
            nc.vector.memset(decpat[:, mi * BL : (mi + 1) * BL], float(DECAY[mi + 1]))

        # h_hat state: m-slices 1..7 only (slice 0 is identically zero)
        hhat = [
            state.tile([128, 7 * BL], bf16, name=f"hhat{u}", tag=f"hhat{u}")
            for u in range(UT)
        ]

        hts = None  # list of 4 [128, BL] bf16 tiles: h(t-1) per u-tile
        hT8 = None  # [128, UT*BL] fp8 copy of h(t-1) for the r-phase matmuls

        def tree8(src, out):
            """out[128,BL] = sum of 8 contiguous BL-slices of src (bf16 tree)."""
            t1 = rtp.tile([128, 4 * BL], bf16, name="rt1", tag="rt1")
            t2 = rtp.tile([128, 2 * BL], bf16, name="rt2", tag="rt2")
            nc.vector.tensor_add(t1[:], src[:, : 4 * BL], src[:, 4 * BL :])
            nc.vector.tensor_add(t2[:], t1[:, : 2 * BL], t1[:, 2 * BL :])
            nc.vector.tensor_add(out[:], t2[:, :BL], t2[:, BL:])

        def tree7(src, out):
            """out[128,BL] = sum of 7 contiguous BL-slices of src (bf16 tree)."""
            t1 = rtp.tile([128, 4 * BL], bf16, name="rt1", tag="rt1")
            t2 = rtp.tile([128, 2 * BL], bf16, name="rt2", tag="rt2")
            nc.vector.tensor_add(t1[:, : 3 * BL], src[:, : 3 * BL], src[:, 4 * BL : 7 * BL])
            nc.vector.tensor_add(t2[:, :BL], t1[:, :BL], t1[:, BL : 2 * BL])
            nc.vector.tensor_add(
                t2[:, BL : 2 * BL], t1[:, 2 * BL : 3 * BL], src[:, 3 * BL : 4 * BL]
            )
            nc.vector.tensor_add(out[:], t2[:, :BL], t2[:, BL : 2 * BL])

        def mm_group_r8(ps, base, xt8_t, hT8, head=True, tail=True):
            """r-phase fused matmul in fp8 DoubleRow: 3 instrs cover K=768."""
            hT8a, hT8b = hT8
            pairs = [
                xt8_t.rearrange("p (two b) -> p two b", two=2),
                hT8a.rearrange("p (two b) -> p two b", two=2),
                hT8b.rearrange("p (two b) -> p two b", two=2),
            ]
            js = ([0, 1] if head else []) + ([2] if tail else [])
            for j in js:
                lhsT = wr_sb[:, base + j * 256 : base + (j + 1) * 256].rearrange(
                    "p (two c) -> p two c", two=2
                )
                nc.tensor.matmul(
                    ps[:], lhsT, pairs[j],
                    start=(j == 0), stop=(j == 2),
                    perf_mode=mybir.MatmulPerfMode.DoubleRow,
                )

        def den_mm(dps, e_tile, m):
            """Accumulate e m-slice into the den psum via an identity matmul."""
            nc.tensor.matmul(
                dps[:], id_sb[:], e_tile[:, m * BL : (m + 1) * BL],
                start=(m == 0), stop=(m == M - 1),
            )

        def mm_group(ps, w_sb, base, rhs, n_k):
            """Accumulate the fused matmul into psum `ps`. rhs: list of K-tile
            moving operands (x k-tiles, then h k-tiles)."""
            for k in range(n_k):
                nc.tensor.matmul(
                    ps[:],
                    w_sb[:, base + k * 128 : base + (k + 1) * 128],
                    rhs[k],
                    start=(k == 0), stop=(k == n_k - 1),
                )

        def emit_y(t, h_tiles):
            psy = py.tile([3, BL], f32, name="ps_y", tag="ps_y")
            for k in range(UT):
                nc.tensor.matmul(
                    psy[:], wo_sb[:, k * 3 : (k + 1) * 3], h_tiles[k][:],
                    start=(k == 0), stop=(k == UT - 1),
                )
            y_sb = stp.tile([3, BL], f32, name="y_sb")
            nc.scalar.copy(y_sb[:], psy[:])
            nc.sync.dma_start(y_d[t], y_sb[:])

        pending_state = []

        def flush_state():
            while pending_state:
                u, etqd, cs_bf, dh0 = pending_state.pop()
                # per-m quanta: small enough that scheduler insertions into
                # the boundary-critical tail cost <0.4us each, not 1.9us
                with tc.high_priority(offset=-150):
                    for mi in range(7):
                        sl = slice(mi * BL, (mi + 1) * BL)
                        nc.vector.tensor_mul(etqd[:, sl], etqd[:, sl], cs_bf[:])
                        nc.vector.tensor_add(hhat[u][:, sl], dh0[:, sl], etqd[:, sl])

        for t in range(T):
            xt_t = xp.tile([128, 2 * BL], bf16, name="xt_t", tag="xt")
            nc.sync.dma_start(xt_t[:], xt_d[t])
            xt8_t = xp.tile([128, 2 * BL], f8, name="xt8_t", tag="xt8")
            nc.sync.dma_start(xt8_t[:], xt8_d[t])
            flush_state()
            rhs_x = [xt_t[:, 0:BL], xt_t[:, BL : 2 * BL]]
            rhs_full = rhs_x + [h[:] for h in hts] if t > 0 else None

                rh_bf = stp.tile([128, UT * BL], bf16, name="rh_bf", bufs=1)
                for u in range(UT):
                    e_r = ep.tile([128, M * BL], bf16, name="e_t", tag="e_t")
                    dps = pden.tile([128, BL], f32, name="ps_den", tag="ps_den")
                    pss = {}
                    if u == 0:
                        # run-ahead: x/h01 pairs of the first 4 groups fill
                        # the PE while u3's elementwise tail finishes h.
                        for m in range(4):
                            pss[m] = pmm.tile([128, BL], f32, name="ps_mm", tag="ps_mm")
                            base = (m * UT + u) * KT * 128
                            mm_group_r8(pss[m], base, xt8_t, hT8, tail=False)
                    for m in range(M):
                        base = (m * UT + u) * KT * 128
                        if m in pss:
                            ps = pss[m]
                            mm_group_r8(ps, base, xt8_t, hT8, head=False)
                        else:
                            ps = pmm.tile([128, BL], f32, name="ps_mm", tag="ps_mm")
                            mm_group_r8(ps, base, xt8_t, hT8)
                        nc.scalar.activation(
                            e_r[:, m * BL : (m + 1) * BL], ps[:],
                            AF.Derivative_Erf, scale=0.125,
                            bias=br_sb[:, u * M + m : u * M + m + 1],
                        )
                        if m >= 2:
                            den_mm(dps, e_r, m - 2)
                    if u == 0:
                        # PE: y-phase of the previous step (hT ready, fills
                        # the pipeline right after the first r-group).
                        emit_y(t - 1, hts)
                    den_mm(dps, e_r, M - 2)
                    den_mm(dps, e_r, M - 1)
                    # DVE: e*h products + weighted-sum tree + normalize
                    eh = tp.tile([128, 7 * BL], bf16, name="ehtq", tag="ehtq", bufs=2)
                    nc.vector.tensor_mul(eh[:], e_r[:, BL:], hhat[u][:])
                    rhn = sp.tile([128, BL], bf16, name="rhn", tag="rhn", bufs=1)
                    tree7(eh, rhn)
                    cr = sp.tile([128, BL], f32, name="crec", tag="crec", bufs=2)
                    nc.vector.reciprocal_approx_fast(out=cr[:], in_=dps[:])
                    nc.vector.tensor_mul(
                        rh_bf[:, u * BL : (u + 1) * BL], rhn[:], cr[:]
                    )            # ---------------- r phase (t=0: h_hat==0 makes r irrelevant) ----
            if t > 0:
                rh_bf = stp.tile([128, UT * BL], bf16, name="rh_bf", bufs=1)
                for u in range(UT):
                    e_r = ep.tile([128, M * BL], bf16, name="e_t", tag="e_t")
                    dps = pden.tile([128, BL], f32, name="ps_den", tag="ps_den")
                    pss = {}
                    if u == 0:
                        # run-ahead: x/h01 pairs of the first 4 groups fill
                        # the PE while u3's elementwise tail finishes h.
                        for m in range(4):
                            pss[m] = pmm.tile([128, BL], f32, name="ps_mm", tag="ps_mm")
                            base = (m * UT + u) * KT * 128
                            mm_group_r8(pss[m], base, xt8_t, hT8, tail=False)
                    for m in range(M):
                        base = (m * UT + u) * KT * 128
                        if m in pss:
                            ps = pss[m]
                            mm_group_r8(ps, base, xt8_t, hT8, head=False)
                        else:
                            ps = pmm.tile([128, BL], f32, name="ps_mm", tag="ps_mm")
                            mm_group_r8(ps, base, xt8_t, hT8)
                        nc.scalar.activation(
                            e_r[:, m * BL : (m + 1) * BL], ps[:],
                            AF.Derivative_Erf, scale=0.125,
                            bias=br_sb[:, u * M + m : u * M + m + 1],
                        )
                        if m >= 2:
                            den_mm(dps, e_r, m - 2)
                    if u == 0:
                        # PE: y-phase of the previous step (hT ready, fills
                        # the pipeline right after the first r-group).
                        emit_y(t - 1, hts)
                    den_mm(dps, e_r, M - 2)
                    den_mm(dps, e_r, M - 1)
                    # DVE: e*h products + weighted-sum tree + normalize
                    eh = tp.tile([128, 7 * BL], bf16, name="ehtq", tag="ehtq", bufs=2)
                    nc.vector.tensor_mul(eh[:], e_r[:, BL:], hhat[u][:])
                    rhn = sp.tile([128, BL], bf16, name="rhn", tag="rhn", bufs=1)
                    tree7(eh, rhn)
                    cr = sp.tile([128, BL], f32, name="crec", tag="crec", bufs=2)
                    nc.vector.reciprocal_approx_fast(out=cr[:], in_=dps[:])
                    nc.vector.tensor_mul(
                        rh_bf[:, u * BL : (u + 1) * BL], rhn[:], cr[:]
                    )

            # ------- s phase matmuls / elementwise, interleaved with q -----
            hts_new = [
                htp.tile([128, BL], bf16, name=f"hT{u}", tag=f"hT{u}")
                for u in range(UT)
            ]
            hT8a_new = htp.tile([128, 2 * BL], f8, name="hT8a", tag="hT8a")
            hT8b_new = htp.tile([128, 2 * BL], f8, name="hT8b", tag="hT8b")

            def s_mm(u, etqd=None, tqd=None, te=None):
                e_s = ep.tile([128, M * BL], bf16, name="e_t", tag="e_t")
                dps = pden.tile([128, BL], f32, name="ps_den", tag="ps_den")
                for m in range(M):
                    wsch = wsp.tile([128, KT * 128], bf16, name="wsch")
                    base = (m * UT + u) * KT * 128
                    nc.sync.dma_start(wsch[:], ws_d[:, base : base + KT * 128])
                    ps = pmm.tile([128, BL], f32, name="ps_mm", tag="ps_mm")
                    rhs = rhs_full if t > 0 else rhs_x
                    mm_group(ps, wsch, 0, rhs, KT if t > 0 else 2)
                    nc.scalar.activation(
                        e_s[:, m * BL : (m + 1) * BL], ps[:],
                        AF.Derivative_Erf,
                        bias=bs_sb[:, u * M + m : u * M + m + 1],
                    )
                    if m >= 2:
                        den_mm(dps, e_s, m - 2)
                    if etqd is not None and m >= 1:
                        sl = slice((m - 1) * BL, m * BL)
                        nc.vector.tensor_mul(
                            etqd[:, sl], e_s[:, m * BL : (m + 1) * BL], tqd[:, sl]
                        )
                        te1, te2 = te
                        if m == 3:
                            nc.vector.tensor_add(
                                te1[:, :BL], etqd[:, :BL], etqd[:, BL : 2 * BL]
                            )
                        elif m == 5:
                            nc.vector.tensor_add(
                                te1[:, BL : 2 * BL],
                                etqd[:, 2 * BL : 3 * BL], etqd[:, 3 * BL : 4 * BL],
                            )
                        elif m == 7:
                            nc.vector.tensor_add(
                                te1[:, 2 * BL : 3 * BL],
                                etqd[:, 4 * BL : 5 * BL], etqd[:, 5 * BL : 6 * BL],
                            )
                            nc.vector.tensor_add(
                                te2[:, :BL], te1[:, :BL], te1[:, BL : 2 * BL]
                            )
                den_mm(dps, e_s, M - 2)
                den_mm(dps, e_s, M - 1)
                return e_s, dps

            def s_el(u, e_s, dps, q_bf, tq, etqd=None, dh=None):
                cs = sp.tile([128, BL], f32, name="crec", tag="crec", bufs=2)
                nc.vector.reciprocal_approx_fast(out=cs[:], in_=dps[:])
                cs_bf = sp.tile([128, BL], bf16, name="csbf", tag="csbf", bufs=2)
                nc.vector.tensor_scalar_mul(cs_bf[:], cs[:], 1.0)
                cs_v = cs_bf.unsqueeze(1).broadcast_to([128, 7, BL])
                if etqd is not None:
                    # fast tail (last u): hT = tree(h*D) + tree(e*tq*D)/den;
                    # only 512-wide ops remain after the last activation.
                    te1, te2, tdh = dh[1]
                    nc.vector.tensor_add(
                        te2[:, BL : 2 * BL], te1[:, 2 * BL : 3 * BL], etqd[:, 6 * BL :]
                    )
                    teE = sp.tile([128, BL], bf16, name="teE", tag="teE", bufs=1)
                    nc.vector.tensor_add(teE[:], te2[:, :BL], te2[:, BL : 2 * BL])
                    sv = sp.tile([128, BL], bf16, name="sv", tag="sv", bufs=1)
                    nc.vector.tensor_mul(sv[:], teE[:], cs_bf[:])
                    nc.vector.tensor_add(hts_new[u][:], tdh[:], sv[:])
                    nc.vector.tensor_scalar_mul(
                        hT8b_new[:, (u % 2) * BL : (u % 2 + 1) * BL],
                        hts_new[u][:], 1.0,
                    )
                    # state update: deferred to the next step's start so the
                    # scheduler cannot slot it into the hT-critical tail.
                    pending_state.append((u, etqd, cs_bf, dh[0]))
                    return
                # s8 = e / den  (softmax gate, broadcast 1/den over m)
                s8 = wpp.tile([128, 7 * BL], bf16, name="wp", tag="wp")
                s8_v = s8.rearrange("p (m b) -> p m b", m=7)
                es_v = e_s[:, BL:].rearrange("p (m b) -> p m b", m=7)
                nc.vector.tensor_tensor(s8_v, es_v, cs_v, op=ALU.mult)
                if t > 0:
                    nc.vector.tensor_mul(s8[:], s8[:], tq[:])  # s*(q-h)
                    nc.vector.tensor_add(tq[:], hhat[u][:], s8[:])  # h + s(q-h)
                    nc.vector.tensor_mul(hhat[u][:], tq[:], decpat[:])
                else:
                    q_v = (
                        q_bf[:, u * BL : (u + 1) * BL]
                        .unsqueeze(1)
                        .broadcast_to([128, 7, BL])
                    )
                    nc.vector.tensor_tensor(s8_v, s8_v, q_v, op=ALU.mult)
                    nc.vector.tensor_mul(hhat[u][:], s8[:], decpat[:])
                tree7(hhat[u], hts_new[u])
                dst = hT8a_new if u < 2 else hT8b_new
                nc.vector.tensor_scalar_mul(
                    dst[:, (u % 2) * BL : (u % 2 + 1) * BL], hts_new[u][:], 1.0
                )

            e_tiles = {0: s_mm(0)}

            # ---------------- q phase ----------------
            q_bf = stp.tile([128, UT * BL], bf16, name="q_bf", bufs=1)
            for uq in range(UT):
                psq = pmm.tile([128, BL], f32, name="ps_mm", tag="ps_mm")
                if t > 0:
                    rhs_q = rhs_x + [
                        rh_bf[:, k * BL : (k + 1) * BL] for k in range(UT)
                    ]
                    mm_group(psq, wq_sb, uq * KT * 128, rhs_q, KT)
                else:
                    mm_group(psq, wq_sb, uq * KT * 128, rhs_x, 2)
                nc.scalar.activation(
                    q_bf[:, uq * BL : (uq + 1) * BL], psq[:],
                    AF.Tanh, bias=bq_sb[:, uq : uq + 1],
                )

            def make_tq(u):
                # DVE: q - h (broadcast q over the 7 m-slices)
                tq = tp.tile([128, 7 * BL], bf16, name="ehtq", tag="ehtq", bufs=2)
                tq_v = tq.rearrange("p (m b) -> p m b", m=7)
                q_v = (
                    q_bf[:, u * BL : (u + 1) * BL]
                    .unsqueeze(1)
                    .broadcast_to([128, 7, BL])
                )
                hh_v = hhat[u].rearrange("p (m b) -> p m b", m=7)
                nc.vector.tensor_tensor(tq_v, q_v, hh_v, op=ALU.subtract)
                return tq

            fast = {}
            preps = {}
            if t > 0:
                # pre-decayed pieces of the last two u-tiles' state updates,
                # emitted right after q where the DVE has early-window slack
                for un in (2, 3):
                    tqn = make_tq(un)
                    tqd = tp.tile([128, 7 * BL], bf16, name="tqd3", tag="tqd3", bufs=2)
                    nc.vector.tensor_mul(tqd[:], tqn[:], decpat[:])
                    dht = tp.tile([128, 7 * BL], bf16, name="dh3", tag="dh3", bufs=2)
                    nc.vector.tensor_mul(dht[:], hhat[un][:], decpat[:])
                    etqd = tp.tile([128, 7 * BL], bf16, name="etqd3", tag="etqd3")
                    tdh = sp.tile([128, BL], bf16, name="tdh", tag="tdh", bufs=2)
                    tree7(dht, tdh)
                    te1 = sp.tile([128, 3 * BL], bf16, name="te1", tag="te1", bufs=1)
                    te2 = sp.tile([128, 2 * BL], bf16, name="te2", tag="te2", bufs=1)
                    fast[un] = (etqd, (dht, (te1, te2, tdh)))
                    preps[un] = (etqd, tqd, (te1, te2))
            for u in range(UT):
                tq = make_tq(u) if (t > 0 and u < 2) else None
                e_s, dps = e_tiles.pop(u)
                fu = fast.get(u)
                s_el(u, e_s, dps, q_bf, tq,
                     etqd=fu[0] if fu else None, dh=fu[1] if fu else None)
                if u + 1 < UT:
                    if t > 0 and u + 1 >= 2:
                        etqd, tqd, te = preps[u + 1]
                        e_tiles[u + 1] = s_mm(u + 1, etqd=etqd, tqd=tqd, te=te)
                    else:
                        e_tiles[u + 1] = s_mm(u + 1)

            hts = hts_new
            hT8 = (hT8a_new, hT8b_new)

        emit_y(T - 1, hts)

    nc.compile()
    _program_cache["nc"] = nc
    return nc


def _prep_shared(W_r, b_r, W_q, b_q, W_s, b_s, W_out):
    import ml_dtypes

    bf = ml_dtypes.bfloat16

    def perm_w(w):  # [768, 4096] -> [128, (m,u,k,c)]
        a = np.ascontiguousarray(w, np.float32).reshape(KT, 128, UT, 128, M)
        return np.ascontiguousarray(
            a.transpose(1, 4, 2, 0, 3).reshape(128, M * UT * KT * 128)
        ).astype(bf)

    from concourse import mybir

    f8np = mybir.dt.np(mybir.dt.float8e4)
    a = np.ascontiguousarray(W_r, np.float32).reshape(KT, 128, UT, 128, M)
    wr = np.ascontiguousarray(
        (a * 8.0).transpose(1, 4, 2, 0, 3).reshape(128, M * UT * KT * 128)
    ).astype(f8np)
    ws = perm_w(W_s)
    wq = np.ascontiguousarray(
        np.asarray(W_q, np.float32)
        .reshape(KT, 128, UT, 128)
        .transpose(1, 2, 0, 3)
        .reshape(128, UT * KT * 128)
    ).astype(bf)
    wo = np.ascontiguousarray(
        np.asarray(W_out, np.float32).reshape(UT, 128, 3).transpose(1, 0, 2).reshape(128, UT * 3)
    ).astype(bf)
    biasr = np.ascontiguousarray(
        (np.asarray(b_r, np.float32).reshape(UT, 128, M) - LN_TAU).transpose(1, 0, 2).reshape(128, UT * M)
    )
    biass = np.ascontiguousarray(
        (np.asarray(b_s, np.float32).reshape(UT, 128, M) - LN_TAU).transpose(1, 0, 2).reshape(128, UT * M)
    )
    biasq = np.ascontiguousarray(np.asarray(b_q, np.float32).reshape(UT, 128).T)
    ident = np.eye(128, dtype=np.float32).astype(bf)
    return dict(wr=wr, ws=ws, wq=wq, wo=wo, biasr=biasr, biass=biass, biasq=biasq,
                ident=ident, _f8np=f8np)


def kernel(x, W_r, b_r, W_q, b_q, W_s, b_s, W_out, b_out):
    _install_axon_hooks_shim()
    from concourse.bass_utils import run_bass_kernel_spmd

    import ml_dtypes

    bf = ml_dtypes.bfloat16

    nc = _build_program()
    shared = _prep_shared(W_r, b_r, W_q, b_q, W_s, b_s, W_out)

    x = np.asarray(x, np.float32)
    in_maps = []
    for c in range(N_CORES):
        xc = x[c * BL : (c + 1) * BL]  # [BL, T, F]
        xtf = np.ascontiguousarray(
            xc.transpose(1, 2, 0).reshape(T, 2, 128, BL).transpose(0, 2, 1, 3).reshape(T, 128, 2 * BL)
        )
        xt = xtf.astype(bf)
        xt8 = xtf.astype(shared["_f8np"])
        m = {k: v for k, v in shared.items() if k != "_f8np"}
        in_maps.append({"xt": xt, "xt8": xt8, **m})

    try:
        res = run_bass_kernel_spmd(nc, in_maps, list(range(N_CORES)))
    except Exception:
        # device pool may be wedged from an earlier crash — reset and retry
        try:
            lib = ctypes.CDLL("/opt/axon/libaxon_pjrt.so")
            lib.axon_reset.restype = ctypes.c_int64
            lib.axon_reset()
        except OSError:
            pass
        res = run_bass_kernel_spmd(nc, in_maps, list(range(N_CORES)))
    _program_cache["last_result"] = res

    out = np.empty((B, T, 3), np.float32)
    for c in range(N_CORES):
        y = res.results[c]["y"]  # [T, 3, BL]
        out[c * BL : (c + 1) * BL] = y.transpose(2, 0, 1)
    return out + np.asarray(b_out, np.float32)
